# revision 3
# baseline (speedup 1.0000x reference)
"""Trainium2 Bass kernel for nn_Attention_83743272337693.

Quantized-attention transformer block:
  q/k/v projections -> RoPE(q,k) -> per-token-per-head int8 quantization of
  q,k -> exact int8 score GEMM -> causal softmax -> attn @ v -> o_proj.

Distribution (8 NeuronCores, SPMD): tensor-parallel over heads. Core c owns
query heads 4c..4c+3 and kv head c (GQA group). Wq/Wk/Wv are sharded
column-wise, Wo row-wise; each core computes a full [S, D] partial of the
output and the host sums the 8 partials (the all-reduce).

Numerics strategy (v2):
- projections run in fp16 (full-rate PE path, 10-bit mantissa inputs); the
  int8 round() decisions flip on ~1% of elements vs the fp32 reference,
  which stays well inside the rel-err budget (simulated rel_l2 ~ 5.8e-3).
- quantized q/k are small integers: exact in bf16, so the score GEMM runs
  bf16 at full rate with fp32 PSUM accumulation. k is pre-scaled by its
  dequant factor rk (bf16), q's factor rq folds into the exp scale.
- softmax avoids the row-max pass entirely: a Cauchy-Schwarz bound
  m_hat = rq*||q_int||*cummax_tile||k_s|| is used as the exp bias. probs
  live in bf16, whose exponent range absorbs the bound's overshoot
  (typical p ~ 1e-7; fp16 would denormal-underflow). scores stay in PSUM
  (no S copy to SBUF); normalization 1/sum folds into the transpose diag.
- attn@v in bf16, o_proj in fp16, per-core output partial in fp16.
- rounding uses the fp32 magic-constant trick (x + 1.5*2^23 - 1.5*2^23),
  matching np.round (round-half-to-even) exactly.
"""
import numpy as np

import concourse.bass as bass
import concourse.mybir as mybir
from concourse import bacc, bass_utils
from concourse.tile import TileContext
from concourse.masks import make_causal_mask, make_identity

# Problem shape (hardcoded per contract).
B, S, D = 1, 2048, 4096
NH, NKV, HD = 32, 8, 128
N_CORES = 8
HQ = NH // N_CORES          # query heads per core (4)
ST = S // 128               # seq tiles (16)
KC = D // 128               # contraction chunks for projections (32)
SCALE = float(HD) ** -0.5
MAGIC = float(np.float32(1.5 * 2 ** 23))
MASK_VAL = -1.0e10

F32 = mybir.dt.float32
BF16 = mybir.dt.bfloat16
F16 = mybir.dt.float16
ADD = mybir.AluOpType.add
SUB = mybir.AluOpType.subtract
MUL = mybir.AluOpType.mult
MAX = mybir.AluOpType.max


def build():
    nc = bacc.Bacc("TRN2", target_bir_lowering=False)

    # Host-prepped layouts (see make_in_maps): per-partition-contiguous.
    xt_d = nc.dram_tensor("xt", [128, ST, KC, 128], F16, kind="ExternalInput")
    cs_d = nc.dram_tensor("cs", [128, 2, ST, HD // 2], F32, kind="ExternalInput")
    wq_d = nc.dram_tensor("wq", [128, KC, HQ * HD], F16, kind="ExternalInput")
    wkv_d = nc.dram_tensor("wkv", [128, KC, 2 * HD], F16, kind="ExternalInput")
    wo_d = nc.dram_tensor("wo", [128, HQ, D], F16, kind="ExternalInput")
    y = nc.dram_tensor("y", [S, D], F16, kind="ExternalOutput")

    with TileContext(nc) as tc:
        with (
            tc.tile_pool(name="persist", bufs=1) as persist,
            tc.tile_pool(name="small", bufs=4) as small,
        ):
            # Persistent SBUF state shared by both phases.
            qT = persist.tile([128, HQ, S], BF16, tag="qT")        # 2 MiB
            kTs = persist.tile([128, S], BF16, tag="kTs")          # 512 KiB
            v_sb = persist.tile([128, ST, HD], BF16, tag="v_sb")   # 512 KiB
            rq_sb = persist.tile([128, HQ, ST], F32, tag="rq_sb")  # exp scale
            nbias = persist.tile([128, HQ, ST], F32, tag="nbias")  # exp bias
            rqn = persist.tile([128, HQ, ST], F32, tag="rqn")      # rq*||q||
            qnsq = persist.tile([128, HQ, ST], F32, tag="qnsq")    # ||q_int||^2
            rkcols = persist.tile([128, ST], F32, tag="rkcols")    # rk per row
            ident_bf = persist.tile([128, 128], BF16, tag="ident_bf")
            ident_f32 = persist.tile([128, 128], F32, tag="ident_f32")
            mask_sb = persist.tile([128, 128], F32, tag="mask_sb")
            ones_sb = persist.tile([1, 128], F32, tag="ones_sb")
            ones_col = persist.tile([128, 1], F32, tag="ones_col")
            rk_bcast = persist.tile([128, S], F32, tag="rk_bcast")  # 1 MiB
            wo_sb = persist.tile([128, HQ, D], F16, tag="wo_sb")    # 4 MiB
            cs_sb = persist.tile([128, 2, ST, HD // 2], F32, tag="cs_sb")

            make_identity(nc, ident_bf[:])
            make_identity(nc, ident_f32[:])
            make_causal_mask(nc, mask_sb[:], mask_val=MASK_VAL)
            nc.gpsimd.memset(ones_sb[:], 1.0)
            nc.gpsimd.memset(ones_col[:], 1.0)

            # ---------------- Phase A: projections + rope + quantize ----------
            with (
                tc.tile_pool(name="wproj", bufs=1) as wpool,
                tc.tile_pool(name="xstream", bufs=3) as xpool,
                tc.tile_pool(name="ropebuf", bufs=2) as rpool,
                tc.tile_pool(name="psA", bufs=2, space="PSUM") as psA,
                tc.tile_pool(name="psT", bufs=2, space="PSUM") as psT,
            ):
                wq_sb = wpool.tile([128, KC, HQ * HD], F16, tag="wq_sb")   # 4 MiB
                wkv_sb = wpool.tile([128, KC, 2 * HD], F16, tag="wkv_sb")  # 2 MiB
                # chunked weight loads so the first projection matmuls can
                # start as soon as their chunk lands (cold-start hiding);
                # cos/sin after the first chunks, wo at the end of the queue.
                for kc4 in range(0, KC, 4):
                    nc.sync.dma_start(wq_sb[:, kc4:kc4 + 4, :],
                                      wq_d.ap()[:, kc4:kc4 + 4, :])
                    nc.sync.dma_start(wkv_sb[:, kc4:kc4 + 4, :],
                                      wkv_d.ap()[:, kc4:kc4 + 4, :])
                    if kc4 == 0:
                        nc.sync.dma_start(cs_sb[:], cs_d.ap())
                nc.sync.dma_start(wo_sb[:], wo_d.ap())

                for st in range(ST):
                    xt = xpool.tile([128, KC, 128], F16, tag="xt")
                    nc.gpsimd.dma_start(xt[:], xt_d.ap()[:, st, :, :])

                    ps_q = psA.tile([128, HQ * HD], F32, tag="ps_q")
                    ps_kv = psA.tile([128, 2 * HD], F32, tag="ps_kv")
                    for kc in range(KC):
                        nc.tensor.matmul(ps_q[:], xt[:, kc, :], wq_sb[:, kc, :],
                                         start=(kc == 0), stop=(kc == KC - 1))
                        nc.tensor.matmul(ps_kv[:], xt[:, kc, :], wkv_sb[:, kc, :],
                                         start=(kc == 0), stop=(kc == KC - 1))

                    # RoPE on q heads + k head (DVE, reads PSUM), batched
                    # across heads via broadcast APs on cos/sin.
                    half = HD // 2
                    cos_t = cs_sb[:, 0, st, :]
                    sin_t = cs_sb[:, 1, st, :]
                    cos_q = cos_t.unsqueeze(1).broadcast_to([128, HQ, half])
                    sin_q = sin_t.unsqueeze(1).broadcast_to([128, HQ, half])
                    # rope layout: [128, HQ+1, HD] (heads 0..3 = q, 4 = k)
                    rope = rpool.tile([128, HQ + 1, HD], F32, tag="rope")
                    tmp = rpool.tile([128, HQ + 1, half], F32, tag="tmp")
                    q_r = ps_q[:].rearrange("p (h t f) -> p h t f", h=HQ, t=2)
                    k_r = ps_kv[:, 0:HD].rearrange("p (t f) -> p t f", t=2)
                    # q heads (batched over h)
                    nc.vector.tensor_tensor(rope[:, :HQ, 0:half], q_r[:, :, 0, :], cos_q, op=MUL)
                    nc.vector.tensor_tensor(tmp[:, :HQ, :], q_r[:, :, 1, :], sin_q, op=MUL)
                    nc.vector.tensor_tensor(rope[:, :HQ, 0:half], rope[:, :HQ, 0:half],
                                            tmp[:, :HQ, :], op=SUB)
                    nc.vector.tensor_tensor(rope[:, :HQ, half:HD], q_r[:, :, 0, :], sin_q, op=MUL)
                    nc.vector.tensor_tensor(tmp[:, :HQ, :], q_r[:, :, 1, :], cos_q, op=MUL)
                    nc.vector.tensor_tensor(rope[:, :HQ, half:HD], rope[:, :HQ, half:HD],
                                            tmp[:, :HQ, :], op=ADD)
                    # k head
                    nc.vector.tensor_tensor(rope[:, HQ, 0:half], k_r[:, 0, :], cos_t, op=MUL)
                    nc.vector.tensor_tensor(tmp[:, HQ, :], k_r[:, 1, :], sin_t, op=MUL)
                    nc.vector.tensor_tensor(rope[:, HQ, 0:half], rope[:, HQ, 0:half],
                                            tmp[:, HQ, :], op=SUB)
                    nc.vector.tensor_tensor(rope[:, HQ, half:HD], k_r[:, 0, :], sin_t, op=MUL)
                    nc.vector.tensor_tensor(tmp[:, HQ, :], k_r[:, 1, :], cos_t, op=MUL)
                    nc.vector.tensor_tensor(rope[:, HQ, half:HD], rope[:, HQ, half:HD],
                                            tmp[:, HQ, :], op=ADD)

                    # v: straight cast to bf16 (no rope/quant).
                    nc.scalar.copy(v_sb[:, st, :], ps_kv[:, HD:2 * HD])

                    # Quantize all 5 rope'd heads at once:
                    # am = absmax, s = 127/clip(am,1e-5), int = round(x*s).
                    am = small.tile([128, HQ + 1], F32, tag="am")
                    nc.vector.tensor_reduce(am[:], rope[:], axis=mybir.AxisListType.X,
                                            op=MAX, apply_absolute_value=True)
                    nc.vector.tensor_scalar_max(am[:], am[:], 1e-5)
                    # dequant factors: rq = am*scale/127 (q heads), rk = am/127
                    nc.vector.tensor_scalar_mul(rq_sb[:, :, st], am[:, 0:HQ],
                                                SCALE / 127.0)
                    nc.vector.tensor_scalar_mul(rkcols[:, st:st + 1],
                                                am[:, HQ:HQ + 1], 1.0 / 127.0)
                    sc = small.tile([128, HQ + 1], F32, tag="sc")
                    nc.vector.reciprocal(sc[:], am[:])
                    nc.vector.tensor_scalar_mul(sc[:], sc[:], 127.0)
                    rnd = rpool.tile([128, HQ + 1, HD], F32, tag="rnd")
                    sc_b = sc[:].unsqueeze(2).broadcast_to([128, HQ + 1, HD])
                    nc.vector.tensor_tensor(rnd[:], rope[:], sc_b, op=MUL)
                    rnd_f = rnd[:].rearrange("p h f -> p (h f)")
                    nc.vector.tensor_scalar(rnd_f, rnd_f, MAGIC, None, op0=ADD)
                    qint = rpool.tile([128, (HQ + 1) * HD], BF16, tag="qint")
                    nc.vector.tensor_scalar(qint[:], rnd_f, -MAGIC, None, op0=ADD)

                    # ||q_int||^2 per q head (for the softmax bound)
                    qint_r = qint[:].rearrange("p (h f) -> p h f", h=HQ + 1)
                    sq = rpool.tile([128, HQ, HD], F32, tag="sq")
                    nc.vector.tensor_tensor(sq[:], qint_r[:, 0:HQ, :],
                                            qint_r[:, 0:HQ, :], op=MUL)
                    nc.vector.tensor_reduce(qnsq[:, :, st], sq[:],
                                            axis=mybir.AxisListType.X, op=ADD)

                    # PE transposes of the quantized heads into [hd, seq] layout.
                    for hh in range(HQ + 1):
                        o = hh * HD
                        ps_t = psT.tile([128, 128], F32, tag="ps_t")
                        nc.tensor.matmul(ps_t[:], qint[:, o:o + HD], ident_bf[:])
                        if hh < HQ:
                            nc.scalar.copy(qT[:, hh, st * 128:(st + 1) * 128], ps_t[:])
                        else:
                            # raw k ints for now; column-scaled by rk below.
                            nc.scalar.copy(kTs[:, st * 128:(st + 1) * 128], ps_t[:])

            # ---------------- rk broadcast + kTs scale + softmax bias ---------
            with (
                tc.tile_pool(name="rkb", bufs=1) as rkpool,
                tc.tile_pool(name="psR", bufs=2, space="PSUM") as psR,
            ):
                # rk per k-row -> [1, S] row -> broadcast to [128, S]
                ps_rt = psR.tile([16, 128], F32, tag="ps_rt")
                nc.tensor.transpose(ps_rt[:], rkcols[:], ident_f32[:])
                rk_rowT = rkpool.tile([16, 128], F32, tag="rk_rowT")
                nc.vector.tensor_copy(rk_rowT[:], ps_rt[:])
                rk_row = rkpool.tile([1, ST, 128], F32, tag="rk_row")
                nc.sync.dma_start(rk_row[:], rk_rowT[:])
                rk_flat = rk_row[:].rearrange("o t s -> o (t s)")
                for b in range(S // 512):
                    ps_b = psR.tile([128, 512], F32, tag="ps_b")
                    nc.tensor.matmul(ps_b[:], ones_sb[:],
                                     rk_flat[:, b * 512:(b + 1) * 512])
                    nc.vector.tensor_copy(rk_bcast[:, b * 512:(b + 1) * 512],
                                          ps_b[:])
                # kTs currently holds raw k ints; scale columns by rk.
                nc.vector.tensor_tensor(kTs[:], kTs[:], rk_bcast[:], op=MUL)

                # ---- softmax bound bias: nbias[:,h,qt] = -rq*||q||*Mk[qt] ----
                # column norms^2 of kTs via ones-matmul of squares
                ksq = rkpool.tile([128, S], F32, tag="ksq")
                nc.vector.tensor_tensor(ksq[:], kTs[:], kTs[:], op=MUL)
                knrm = rkpool.tile([1, S], F32, tag="knrm")
                for b in range(S // 512):
                    ps_n = psR.tile([1, 512], F32, tag="ps_n")
                    nc.tensor.matmul(ps_n[:], ones_col[:],
                                     ksq[:, b * 512:(b + 1) * 512])
                    nc.scalar.sqrt(knrm[:, b * 512:(b + 1) * 512], ps_n[:])
                # per-tile max of ||k_s||, then running max over tiles
                mk = rkpool.tile([1, ST], F32, tag="mk")
                nc.vector.tensor_reduce(
                    mk[:], knrm[:].rearrange("o (t s) -> o t s", t=ST),
                    axis=mybir.AxisListType.X, op=MAX)
                mkr = rkpool.tile([1, ST], F32, tag="mkr")
                nc.vector.tensor_tensor_scan(
                    mkr[:], mk[:], mk[:], initial=0.0, op0=MAX, op1=MAX)
                # broadcast running max to all partitions
                ps_mk = psR.tile([128, ST], F32, tag="ps_mk")
                nc.tensor.matmul(ps_mk[:], ones_sb[:], mkr[:])
                mk_b = rkpool.tile([128, ST], F32, tag="mk_b")
                nc.vector.tensor_copy(mk_b[:], ps_mk[:])
                # rqn = rq * ||q_int||  (||q|| = sqrt(qnsq))
                qn = rkpool.tile([128, HQ, ST], F32, tag="qn")
                nc.scalar.sqrt(qn[:], qnsq[:])
                nc.vector.tensor_tensor(rqn[:], rq_sb[:], qn[:], op=MUL)
                # nbias = -(rqn * Mk[qt])
                mk_q = mk_b[:].unsqueeze(1).broadcast_to([128, HQ, ST])
                nc.vector.scalar_tensor_tensor(
                    nbias[:], rqn[:], -1.0, mk_q, op0=MUL, op1=MUL)

            # ---------------- Phase B: attention + o_proj -----------------
            with (
                tc.tile_pool(name="sbp", bufs=2) as sbp,
                tc.tile_pool(name="pbuf", bufs=2) as pbuf,
                tc.tile_pool(name="obuf", bufs=3) as obuf,
                tc.tile_pool(name="psS", bufs=1, space="PSUM") as psS,
                tc.tile_pool(name="psP", bufs=2, space="PSUM") as psP,
                tc.tile_pool(name="psV", bufs=1, space="PSUM") as psV,
                tc.tile_pool(name="psO", bufs=2, space="PSUM") as psO,
            ):
                for qt in range(ST):
                    K = (qt + 1) * 128
                    pT = pbuf.tile([128, HQ, S], BF16, tag="pT")
                    for h in range(HQ):
                        lhs_q = qT[:, h, qt * 128:(qt + 1) * 128]
                        p_sb = sbp.tile([128, S], BF16, tag="p_sb")
                        ssum = small.tile([128, 2], F32, tag="ssum")
                        rq_h = rq_sb[:, h, qt:qt + 1]
                        nb_h = nbias[:, h, qt:qt + 1]
                        # process the row in <=1536-wide pieces, PSUM-resident
                        pieces = [(0, K)] if K <= 1536 else [(0, 1536), (1536, K)]
                        for pi, (p0, p1) in enumerate(pieces):
                            pw = p1 - p0
                            ps_S = psS.tile([128, 1536], F32, tag="ps_S")
                            for n0 in range(0, pw, 512):
                                w_ = min(pw, n0 + 512) - n0
                                nc.tensor.matmul(ps_S[:, n0:n0 + w_], lhs_q,
                                                 kTs[:, p0 + n0:p0 + n0 + w_])
                            if p1 == K:
                                # causal mask on the diagonal block
                                db = qt * 128 - p0
                                nc.vector.tensor_tensor(
                                    ps_S[:, db:db + 128], ps_S[:, db:db + 128],
                                    mask_sb[:], op=ADD)
                            nc.scalar.activation(
                                p_sb[:, p0:p1], ps_S[:, 0:pw],
                                mybir.ActivationFunctionType.Exp,
                                bias=nb_h, scale=rq_h,
                                accum_out=ssum[:, pi:pi + 1])
                        if len(pieces) == 2:
                            nc.vector.tensor_tensor(ssum[:, 0:1], ssum[:, 0:1],
                                                    ssum[:, 1:2], op=ADD)
                        w = small.tile([128, 1], F32, tag="w")
                        nc.vector.reciprocal(w[:], ssum[:, 0:1])
                        diag = pbuf.tile([128, 128], BF16, tag="diag")
                        nc.vector.tensor_scalar_mul(diag[:], ident_bf[:], w[:])
                        # p^T (scaled by 1/sum) via PE, two blocks per bank
                        for kc in range(0, qt + 1, 2):
                            kn = min(2, qt + 1 - kc)
                            ps_p = psP.tile([128, 256], F32, tag="ps_p")
                            for j in range(kn):
                                nc.tensor.matmul(
                                    ps_p[:, j * 128:(j + 1) * 128],
                                    p_sb[:, (kc + j) * 128:(kc + j + 1) * 128],
                                    diag[:])
                            if kc % 8 < 6:
                                nc.vector.tensor_copy(
                                    pT[:, h, kc * 128:(kc + kn) * 128],
                                    ps_p[:, :kn * 128])
                            else:
                                nc.scalar.copy(
                                    pT[:, h, kc * 128:(kc + kn) * 128],
                                    ps_p[:, :kn * 128])
                    # attn @ v for all 4 heads at once (N=512 moving)
                    ps_oh = psV.tile([128, HQ * 128], F32, tag="ps_oh")
                    for kc in range(qt + 1):
                        nc.tensor.matmul(
                            ps_oh[:], v_sb[:, kc, :],
                            pT[:, :, kc * 128:(kc + 1) * 128],
                            start=(kc == 0), stop=(kc == qt))
                    ohT = pbuf.tile([128, HQ * 128], F16, tag="ohT")
                    nc.scalar.copy(ohT[:], ps_oh[:])
                    # o_proj for this q-tile: accumulate the 4 heads.
                    for b in range(D // 512):
                        ps_O = psO.tile([128, 512], F32, tag="ps_O")
                        for h in range(HQ):
                            nc.tensor.matmul(
                                ps_O[:], ohT[:, h * 128:(h + 1) * 128],
                                wo_sb[:, h, b * 512:(b + 1) * 512],
                                start=(h == 0), stop=(h == HQ - 1))
                        out_t = obuf.tile([128, 512], F16, tag="out_t")
                        if b % 2 == 0:
                            nc.vector.tensor_copy(out_t[:], ps_O[:])
                        else:
                            nc.scalar.copy(out_t[:], ps_O[:])
                        nc.gpsimd.dma_start(
                            y.ap()[qt * 128:(qt + 1) * 128,
                                   b * 512:(b + 1) * 512], out_t[:])

    nc.finalize()
    return nc


_NC_CACHE = None


def _get_nc():
    global _NC_CACHE
    if _NC_CACHE is None:
        _NC_CACHE = build()
    return _NC_CACHE


def make_in_maps(x, cos, sin, Wq, Wk, Wv, Wo):
    """Shard the full inputs into the 8 per-core input maps."""
    x = np.asarray(x, np.float32).reshape(S, D)
    # xt[p, st, kc, s] = x[st*128+s, kc*128+p]  (fp16)
    xt = np.ascontiguousarray(
        x.reshape(ST, 128, KC, 128).transpose(3, 0, 2, 1)).astype(np.float16)
    # cs[p, {cos,sin}, st, f] = {cos,sin}[st*128+p, f]
    cosr = np.asarray(cos, np.float32).reshape(ST, 128, HD // 2)
    sinr = np.asarray(sin, np.float32).reshape(ST, 128, HD // 2)
    cs = np.ascontiguousarray(
        np.stack([cosr, sinr], axis=0).transpose(2, 0, 1, 3))
    Wq = np.asarray(Wq, np.float32)
    Wk = np.asarray(Wk, np.float32)
    Wv = np.asarray(Wv, np.float32)
    Wo = np.asarray(Wo, np.float32)
    in_maps = []
    for c in range(N_CORES):
        qs = slice(c * HQ * HD, (c + 1) * HQ * HD)
        ks = slice(c * HD, (c + 1) * HD)
        # wq[p, kc, n] = Wq[kc*128+p, qs][n]
        wq = np.ascontiguousarray(
            Wq[:, qs].reshape(KC, 128, HQ * HD).transpose(1, 0, 2)
        ).astype(np.float16)
        wkv_full = np.concatenate([Wk[:, ks], Wv[:, ks]], axis=1)
        wkv = np.ascontiguousarray(
            wkv_full.reshape(KC, 128, 2 * HD).transpose(1, 0, 2)
        ).astype(np.float16)
        # wo[p, h, n] = Wo[qs][h*128+p, n]
        wo = np.ascontiguousarray(
            Wo[qs, :].reshape(HQ, 128, D).transpose(1, 0, 2)
        ).astype(np.float16)
        in_maps.append({"xt": xt, "cs": cs, "wq": wq, "wkv": wkv, "wo": wo})
    return in_maps


def run(x, cos, sin, Wq, Wk, Wv, Wo, trace=False):
    nc = _get_nc()
    in_maps = make_in_maps(x, cos, sin, Wq, Wk, Wv, Wo)
    res = bass_utils.run_bass_kernel_spmd(
        nc, in_maps, core_ids=list(range(N_CORES)), trace=trace)
    partials = np.stack([res.results[c]["y"] for c in range(N_CORES)])
    out = partials.astype(np.float64).sum(axis=0).astype(np.float32)
    return out.reshape(B, S, D), res


def kernel(x, cos, sin, Wq, Wk, Wv, Wo):
    out, _ = run(x, cos, sin, Wq, Wk, Wv, Wo, trace=False)
    return out


# revision 8
# speedup vs baseline: 1.0250x; 1.0250x over previous
"""Trainium2 Bass kernel for nn_Attention_83743272337693.

Quantized-attention transformer block:
  q/k/v projections -> RoPE(q,k) -> per-token-per-head int8 quantization of
  q,k -> exact int8 score GEMM -> causal softmax -> attn @ v -> o_proj.

Distribution (8 NeuronCores, SPMD): tensor-parallel over heads. Core c owns
query heads 4c..4c+3 and kv head c (GQA group). Wq/Wk/Wv are sharded
column-wise, Wo row-wise; each core computes a full [S, D] partial of the
output and the host sums the 8 partials (the all-reduce).

Numerics strategy (v3):
- projections run in fp16 (full-rate PE path, 10-bit mantissa inputs); the
  int8 round() decisions flip on ~1% of elements vs the fp32 reference,
  which stays well inside the rel-err budget (simulated rel_l2 ~ 5.8e-3,
  HW-measured 6.1e-3 vs tolerance 2e-2).
- quantized q/k are small integers: exact in bf16, so the score GEMM runs
  bf16 at full rate with fp32 PSUM accumulation. k is pre-scaled by its
  dequant factor rk (bf16), q's factor rq folds into the exp scale.
- softmax avoids the row-max pass entirely: a Cauchy-Schwarz bound
  m_hat = rq*||q_int||*cummax_tile||k_s|| is used as the exp bias. probs
  live in bf16, whose exponent range absorbs the bound's overshoot
  (typical p ~ 1e-7; fp16 would denormal-underflow). scores stay in PSUM
  in 512-wide chunks (no S copy to SBUF; chunked exp with a shared bias);
  normalization 1/sum folds into the transpose diag.
- attn@v in bf16, o_proj in fp16, per-core output partial in fp16.
- rounding uses the fp32 magic-constant trick (x + 1.5*2^23 - 1.5*2^23),
  matching np.round (round-half-to-even) exactly.
"""
import numpy as np

import concourse.bass as bass
import concourse.mybir as mybir
from concourse import bacc, bass_utils
from concourse.tile import TileContext
from concourse.masks import make_causal_mask, make_identity

# Problem shape (hardcoded per contract).
B, S, D = 1, 2048, 4096
NH, NKV, HD = 32, 8, 128
N_CORES = 8
HQ = NH // N_CORES          # query heads per core (4)
ST = S // 128               # seq tiles (16)
KC = D // 128               # contraction chunks for projections (32)
SCALE = float(HD) ** -0.5
MAGIC = float(np.float32(1.5 * 2 ** 23))
MASK_VAL = -1.0e10

F32 = mybir.dt.float32
BF16 = mybir.dt.bfloat16
F16 = mybir.dt.float16
ADD = mybir.AluOpType.add
SUB = mybir.AluOpType.subtract
MUL = mybir.AluOpType.mult
MAX = mybir.AluOpType.max
Ident = mybir.ActivationFunctionType.Identity


def build():
    nc = bacc.Bacc("TRN2", target_bir_lowering=False)

    # Host-prepped layouts (see make_in_maps): per-partition-contiguous.
    xt_d = nc.dram_tensor("xt", [128, ST, KC, 128], F16, kind="ExternalInput")
    cs_d = nc.dram_tensor("cs", [128, 2, ST, HD // 2], F32, kind="ExternalInput")
    wq_d = nc.dram_tensor("wq", [128, KC, HQ * HD], F16, kind="ExternalInput")
    wkv_d = nc.dram_tensor("wkv", [128, KC, 2 * HD], F16, kind="ExternalInput")
    wo_d = nc.dram_tensor("wo", [128, HQ, D], F16, kind="ExternalInput")
    y = nc.dram_tensor("y", [S, D], F16, kind="ExternalOutput")

    with TileContext(nc) as tc:
        with (
            tc.tile_pool(name="persist", bufs=1) as persist,
            tc.tile_pool(name="small", bufs=4) as small,
        ):
            # Persistent SBUF state shared by both phases.
            qT = persist.tile([128, HQ, S], BF16, tag="qT")        # 2 MiB
            kTs = persist.tile([128, S], BF16, tag="kTs")          # 512 KiB
            v_sb = persist.tile([128, ST, HD], BF16, tag="v_sb")   # 512 KiB
            rq_sb = persist.tile([128, HQ, ST], F32, tag="rq_sb")  # exp scale
            nbias = persist.tile([128, HQ, ST], F32, tag="nbias")  # exp bias
            qnsq = persist.tile([128, HQ, ST], F32, tag="qnsq")    # ||q_int||^2
            mk2 = persist.tile([1, ST], F32, tag="mk2")            # max||k_s||^2/tile
            ident_bf = persist.tile([128, 128], BF16, tag="ident_bf")
            ident_f32 = persist.tile([128, 128], F32, tag="ident_f32")
            mask_sb = persist.tile([128, 128], F32, tag="mask_sb")
            ones_sb = persist.tile([1, 128], F32, tag="ones_sb")
            ones_col = persist.tile([128, 1], F32, tag="ones_col")
            magic_p = persist.tile([128, 1], F32, tag="magic_p")
            magic_n = persist.tile([128, 1], F32, tag="magic_n")
            wo_sb = persist.tile([128, HQ, D], F16, tag="wo_sb")    # 4 MiB
            cs_sb = persist.tile([128, 2, ST, HD // 2], F32, tag="cs_sb")

            make_identity(nc, ident_bf[:])
            make_identity(nc, ident_f32[:])
            make_causal_mask(nc, mask_sb[:], mask_val=MASK_VAL)
            nc.gpsimd.memset(ones_sb[:], 1.0)
            nc.gpsimd.memset(ones_col[:], 1.0)
            nc.gpsimd.memset(magic_p[:], MAGIC)
            nc.gpsimd.memset(magic_n[:], -MAGIC)

            # ---------------- Phase A: projections + rope + quantize ----------
            with (
                tc.tile_pool(name="wproj", bufs=1) as wpool,
                tc.tile_pool(name="xstream", bufs=3) as xpool,
                tc.tile_pool(name="ropebuf", bufs=2) as rpool,
                tc.tile_pool(name="psA", bufs=2, space="PSUM") as psA,
                tc.tile_pool(name="psT", bufs=3, space="PSUM") as psT,
            ):
                wq_sb = wpool.tile([128, KC, HQ * HD], F16, tag="wq_sb")   # 4 MiB
                wkv_sb = wpool.tile([128, KC, 2 * HD], F16, tag="wkv_sb")  # 2 MiB
                # chunked weight loads, smallest first, so the first projection
                # matmuls start as soon as their chunk lands; wo queued last.
                wsched = [(0, 1), (1, 2), (2, 4)] + [(k, k + 4) for k in range(4, KC, 4)]
                for i, (a, b) in enumerate(wsched):
                    nc.sync.dma_start(wq_sb[:, a:b, :], wq_d.ap()[:, a:b, :])
                    nc.sync.dma_start(wkv_sb[:, a:b, :], wkv_d.ap()[:, a:b, :])
                    if i == 2:
                        nc.sync.dma_start(cs_sb[:], cs_d.ap())
                nc.sync.dma_start(wo_sb[:], wo_d.ap())

                for st in range(ST):
                    xt = xpool.tile([128, KC, 128], F16, tag="xt")
                    for kq in range(0, KC, 8):
                        nc.gpsimd.dma_start(xt[:, kq:kq + 8, :],
                                            xt_d.ap()[:, st, kq:kq + 8, :])

                    ps_q = psA.tile([128, HQ * HD], F32, tag="ps_q")
                    ps_kv = psA.tile([128, 2 * HD], F32, tag="ps_kv")
                    for kc in range(KC):
                        nc.tensor.matmul(ps_q[:], xt[:, kc, :], wq_sb[:, kc, :],
                                         start=(kc == 0), stop=(kc == KC - 1))
                        nc.tensor.matmul(ps_kv[:], xt[:, kc, :], wkv_sb[:, kc, :],
                                         start=(kc == 0), stop=(kc == KC - 1))

                    # RoPE on q heads + k head (DVE, reads PSUM), batched
                    # across heads via broadcast APs on cos/sin.
                    half = HD // 2
                    cos_t = cs_sb[:, 0, st, :]
                    sin_t = cs_sb[:, 1, st, :]
                    cos_q = cos_t.unsqueeze(1).broadcast_to([128, HQ, half])
                    sin_q = sin_t.unsqueeze(1).broadcast_to([128, HQ, half])
                    # rope layout: [128, HQ+1, HD] (heads 0..3 = q, 4 = k)
                    rope = rpool.tile([128, HQ + 1, HD], F32, tag="rope")
                    tmp = rpool.tile([128, HQ + 1, half], F32, tag="tmp")
                    q_r = ps_q[:].rearrange("p (h t f) -> p h t f", h=HQ, t=2)
                    k_r = ps_kv[:, 0:HD].rearrange("p (t f) -> p t f", t=2)
                    # q heads (batched over h)
                    nc.vector.tensor_tensor(rope[:, :HQ, 0:half], q_r[:, :, 0, :], cos_q, op=MUL)
                    nc.vector.tensor_tensor(tmp[:, :HQ, :], q_r[:, :, 1, :], sin_q, op=MUL)
                    nc.vector.tensor_tensor(rope[:, :HQ, 0:half], rope[:, :HQ, 0:half],
                                            tmp[:, :HQ, :], op=SUB)
                    nc.vector.tensor_tensor(rope[:, :HQ, half:HD], q_r[:, :, 0, :], sin_q, op=MUL)
                    nc.vector.tensor_tensor(tmp[:, :HQ, :], q_r[:, :, 1, :], cos_q, op=MUL)
                    nc.vector.tensor_tensor(rope[:, :HQ, half:HD], rope[:, :HQ, half:HD],
                                            tmp[:, :HQ, :], op=ADD)
                    # k head
                    nc.vector.tensor_tensor(rope[:, HQ, 0:half], k_r[:, 0, :], cos_t, op=MUL)
                    nc.vector.tensor_tensor(tmp[:, HQ, :], k_r[:, 1, :], sin_t, op=MUL)
                    nc.vector.tensor_tensor(rope[:, HQ, 0:half], rope[:, HQ, 0:half],
                                            tmp[:, HQ, :], op=SUB)
                    nc.vector.tensor_tensor(rope[:, HQ, half:HD], k_r[:, 0, :], sin_t, op=MUL)
                    nc.vector.tensor_tensor(tmp[:, HQ, :], k_r[:, 1, :], cos_t, op=MUL)
                    nc.vector.tensor_tensor(rope[:, HQ, half:HD], rope[:, HQ, half:HD],
                                            tmp[:, HQ, :], op=ADD)

                    # v: straight cast to bf16 (no rope/quant).
                    nc.scalar.copy(v_sb[:, st, :], ps_kv[:, HD:2 * HD])

                    # Quantize all 5 rope'd heads at once:
                    # am = absmax, s = 127/clip(am,1e-5), int = round(x*s).
                    am = small.tile([128, HQ + 1], F32, tag="am")
                    nc.vector.tensor_reduce(am[:], rope[:], axis=mybir.AxisListType.X,
                                            op=MAX, apply_absolute_value=True)
                    nc.vector.tensor_scalar_max(am[:], am[:], 1e-5)
                    # dequant factors: rq = am*scale/127 (q heads), rk = am/127
                    nc.vector.tensor_scalar_mul(rq_sb[:, :, st], am[:, 0:HQ],
                                                SCALE / 127.0)
                    rk_col = small.tile([128, 1], F32, tag="rk_col")
                    nc.vector.tensor_scalar_mul(rk_col[:], am[:, HQ:HQ + 1],
                                                1.0 / 127.0)
                    sc = small.tile([128, HQ + 1], F32, tag="sc")
                    nc.vector.reciprocal(sc[:], am[:])
                    nc.vector.tensor_scalar_mul(sc[:], sc[:], 127.0)
                    rnd = rpool.tile([128, HQ + 1, HD], F32, tag="rnd")
                    sc_b = sc[:].unsqueeze(2).broadcast_to([128, HQ + 1, HD])
                    nc.gpsimd.tensor_tensor(rnd[:], rope[:], sc_b, op=MUL)
                    rnd_f = rnd[:].rearrange("p h f -> p (h f)")
                    # round via magic constant; the two adds run on ACT
                    nc.scalar.activation(rnd_f, rnd_f, Ident, bias=magic_p[:])
                    qint = rpool.tile([128, (HQ + 1) * HD], BF16, tag="qint")
                    nc.scalar.activation(qint[:], rnd_f, Ident, bias=magic_n[:])

                    # ||q_int||^2 per q head (for the softmax bound)
                    qint_r = qint[:].rearrange("p (h f) -> p h f", h=HQ + 1)
                    sq = rpool.tile([128, HQ, HD], F32, tag="sq")
                    nc.gpsimd.tensor_tensor(sq[:], qint_r[:, 0:HQ, :],
                                            qint_r[:, 0:HQ, :], op=MUL)
                    nc.vector.tensor_reduce(qnsq[:, :, st], sq[:],
                                            axis=mybir.AxisListType.X, op=ADD)

                    # rk broadcast for this tile: [128,1] -> [1,128] -> [128,128]
                    ps_r1 = psT.tile([1, 128], F32, tag="ps_t")
                    nc.tensor.matmul(ps_r1[:], rk_col[:], ident_f32[:])
                    rk1 = small.tile([1, 128], F32, tag="rk1")
                    nc.vector.tensor_copy(rk1[:], ps_r1[:])
                    ps_rb = psT.tile([128, 128], F32, tag="ps_t")
                    nc.tensor.matmul(ps_rb[:], ones_sb[:], rk1[:])
                    rkb = rpool.tile([128, 128], F32, tag="rkb")
                    nc.vector.tensor_copy(rkb[:], ps_rb[:])

                    # PE transposes of the quantized heads into [hd, seq] layout.
                    sb = slice(st * 128, (st + 1) * 128)
                    for hh in range(HQ + 1):
                        o = hh * HD
                        ps_t = psT.tile([128, 128], F32, tag="ps_t")
                        nc.tensor.matmul(ps_t[:], qint[:, o:o + HD], ident_bf[:])
                        if hh < HQ:
                            nc.scalar.copy(qT[:, hh, sb], ps_t[:])
                        else:
                            # scale k^T columns by rk while copying out
                            nc.vector.tensor_tensor(kTs[:, sb], ps_t[:], rkb[:],
                                                    op=MUL)
                    # max column norm^2 of this kTs block (softmax bound)
                    ksq = rpool.tile([128, 128], F32, tag="ksq")
                    nc.vector.tensor_tensor(ksq[:], kTs[:, sb], kTs[:, sb], op=MUL)
                    ps_n = psT.tile([1, 128], F32, tag="ps_t")
                    nc.tensor.matmul(ps_n[:], ones_col[:], ksq[:])
                    nc.vector.tensor_reduce(mk2[:, st:st + 1], ps_n[:],
                                            axis=mybir.AxisListType.X, op=MAX)

            # ---------------- softmax bound bias (tiny) -----------------------
            with (
                tc.tile_pool(name="rkb2", bufs=1) as rkpool,
                tc.tile_pool(name="psR", bufs=2, space="PSUM") as psR,
            ):
                # cummax over tiles, sqrt, broadcast to all partitions
                mk2r = rkpool.tile([1, ST], F32, tag="mk2r")
                nc.vector.tensor_tensor_scan(
                    mk2r[:], mk2[:], mk2[:], initial=0.0, op0=MAX, op1=MAX)
                mkr = rkpool.tile([1, ST], F32, tag="mkr")
                nc.scalar.sqrt(mkr[:], mk2r[:])
                ps_mk = psR.tile([128, ST], F32, tag="ps_mk")
                nc.tensor.matmul(ps_mk[:], ones_sb[:], mkr[:])
                mk_b = rkpool.tile([128, ST], F32, tag="mk_b")
                nc.vector.tensor_copy(mk_b[:], ps_mk[:])
                # nbias = -(rq*||q||) * Mk[qt]
                qn = rkpool.tile([128, HQ, ST], F32, tag="qn")
                nc.scalar.sqrt(qn[:], qnsq[:])
                rqn = rkpool.tile([128, HQ, ST], F32, tag="rqn")
                nc.vector.tensor_tensor(rqn[:], rq_sb[:], qn[:], op=MUL)
                mk_q = mk_b[:].unsqueeze(1).broadcast_to([128, HQ, ST])
                nc.vector.scalar_tensor_tensor(
                    nbias[:], rqn[:], -1.0, mk_q, op0=MUL, op1=MUL)

            # ---------------- Phase B: attention + o_proj -----------------
            with (
                tc.tile_pool(name="sbp", bufs=2) as sbp,
                tc.tile_pool(name="pbuf", bufs=2) as pbuf,
                tc.tile_pool(name="obuf", bufs=3) as obuf,
                tc.tile_pool(name="psS", bufs=3, space="PSUM") as psS,
                tc.tile_pool(name="psP", bufs=2, space="PSUM") as psP,
                tc.tile_pool(name="psV", bufs=1, space="PSUM") as psV,
                tc.tile_pool(name="psO", bufs=2, space="PSUM") as psO,
            ):
                for qt in range(ST):
                    K = (qt + 1) * 128
                    nchunk = (K + 511) // 512
                    pT = pbuf.tile([128, HQ, S], BF16, tag="pT")
                    for h in range(HQ):
                        lhs_q = qT[:, h, qt * 128:(qt + 1) * 128]
                        p_sb = sbp.tile([128, S], BF16, tag="p_sb")
                        ssum = small.tile([128, 4], F32, tag="ssum")
                        rq_h = rq_sb[:, h, qt:qt + 1]
                        nb_h = nbias[:, h, qt:qt + 1]
                        # 512-wide PSUM-resident chunks, shared global bias
                        for ci in range(nchunk):
                            c0 = ci * 512
                            cw = min(K, c0 + 512) - c0
                            ps_S = psS.tile([128, 512], F32, tag="ps_S")
                            nc.tensor.matmul(ps_S[:, 0:cw], lhs_q,
                                             kTs[:, c0:c0 + cw])
                            if ci == nchunk - 1:
                                # causal mask on the diagonal block
                                db = qt * 128 - c0
                                nc.vector.tensor_tensor(
                                    ps_S[:, db:db + 128], ps_S[:, db:db + 128],
                                    mask_sb[:], op=ADD)
                            nc.scalar.activation(
                                p_sb[:, c0:c0 + cw], ps_S[:, 0:cw],
                                mybir.ActivationFunctionType.Exp,
                                bias=nb_h, scale=rq_h,
                                accum_out=ssum[:, ci:ci + 1])
                        w = small.tile([128, 1], F32, tag="w")
                        if nchunk > 1:
                            tot = small.tile([128, 1], F32, tag="tot")
                            nc.vector.tensor_reduce(tot[:], ssum[:, 0:nchunk],
                                                    axis=mybir.AxisListType.X,
                                                    op=ADD)
                            nc.vector.reciprocal(w[:], tot[:])
                        else:
                            nc.vector.reciprocal(w[:], ssum[:, 0:1])
                        diag = pbuf.tile([128, 128], BF16, tag="diag")
                        nc.vector.tensor_scalar_mul(diag[:], ident_bf[:], w[:])
                        # p^T (scaled by 1/sum) via PE, two blocks per bank
                        for kc in range(0, qt + 1, 2):
                            kn = min(2, qt + 1 - kc)
                            ps_p = psP.tile([128, 256], F32, tag="ps_p")
                            for j in range(kn):
                                nc.tensor.matmul(
                                    ps_p[:, j * 128:(j + 1) * 128],
                                    p_sb[:, (kc + j) * 128:(kc + j + 1) * 128],
                                    diag[:])
                            if kc % 4 < 2:
                                nc.vector.tensor_copy(
                                    pT[:, h, kc * 128:(kc + kn) * 128],
                                    ps_p[:, :kn * 128])
                            else:
                                nc.scalar.copy(
                                    pT[:, h, kc * 128:(kc + kn) * 128],
                                    ps_p[:, :kn * 128])
                    # attn @ v for all 4 heads at once (N=512 moving)
                    ps_oh = psV.tile([128, HQ * 128], F32, tag="ps_oh")
                    for kc in range(qt + 1):
                        nc.tensor.matmul(
                            ps_oh[:], v_sb[:, kc, :],
                            pT[:, :, kc * 128:(kc + 1) * 128],
                            start=(kc == 0), stop=(kc == qt))
                    ohT = pbuf.tile([128, HQ * 128], F16, tag="ohT")
                    nc.scalar.copy(ohT[:], ps_oh[:])
                    # o_proj for this q-tile: accumulate the 4 heads.
                    for b in range(D // 512):
                        ps_O = psO.tile([128, 512], F32, tag="ps_O")
                        for h in range(HQ):
                            nc.tensor.matmul(
                                ps_O[:], ohT[:, h * 128:(h + 1) * 128],
                                wo_sb[:, h, b * 512:(b + 1) * 512],
                                start=(h == 0), stop=(h == HQ - 1))
                        out_t = obuf.tile([128, 512], F16, tag="out_t")
                        if b % 2 == 0:
                            nc.vector.tensor_copy(out_t[:], ps_O[:])
                        else:
                            nc.scalar.copy(out_t[:], ps_O[:])
                        nc.gpsimd.dma_start(
                            y.ap()[qt * 128:(qt + 1) * 128,
                                   b * 512:(b + 1) * 512], out_t[:])

    nc.finalize()
    return nc


_NC_CACHE = None


def _get_nc():
    global _NC_CACHE
    if _NC_CACHE is None:
        _NC_CACHE = build()
    return _NC_CACHE


def make_in_maps(x, cos, sin, Wq, Wk, Wv, Wo):
    """Shard the full inputs into the 8 per-core input maps."""
    x = np.asarray(x, np.float32).reshape(S, D)
    # xt[p, st, kc, s] = x[st*128+s, kc*128+p]  (fp16)
    xt = np.ascontiguousarray(
        x.reshape(ST, 128, KC, 128).transpose(3, 0, 2, 1)).astype(np.float16)
    # cs[p, {cos,sin}, st, f] = {cos,sin}[st*128+p, f]
    cosr = np.asarray(cos, np.float32).reshape(ST, 128, HD // 2)
    sinr = np.asarray(sin, np.float32).reshape(ST, 128, HD // 2)
    cs = np.ascontiguousarray(
        np.stack([cosr, sinr], axis=0).transpose(2, 0, 1, 3))
    Wq = np.asarray(Wq, np.float32)
    Wk = np.asarray(Wk, np.float32)
    Wv = np.asarray(Wv, np.float32)
    Wo = np.asarray(Wo, np.float32)
    in_maps = []
    for c in range(N_CORES):
        qs = slice(c * HQ * HD, (c + 1) * HQ * HD)
        ks = slice(c * HD, (c + 1) * HD)
        # wq[p, kc, n] = Wq[kc*128+p, qs][n]
        wq = np.ascontiguousarray(
            Wq[:, qs].reshape(KC, 128, HQ * HD).transpose(1, 0, 2)
        ).astype(np.float16)
        wkv_full = np.concatenate([Wk[:, ks], Wv[:, ks]], axis=1)
        wkv = np.ascontiguousarray(
            wkv_full.reshape(KC, 128, 2 * HD).transpose(1, 0, 2)
        ).astype(np.float16)
        # wo[p, h, n] = Wo[qs][h*128+p, n]
        wo = np.ascontiguousarray(
            Wo[qs, :].reshape(HQ, 128, D).transpose(1, 0, 2)
        ).astype(np.float16)
        in_maps.append({"xt": xt, "cs": cs, "wq": wq, "wkv": wkv, "wo": wo})
    return in_maps


def run(x, cos, sin, Wq, Wk, Wv, Wo, trace=False):
    nc = _get_nc()
    in_maps = make_in_maps(x, cos, sin, Wq, Wk, Wv, Wo)
    res = bass_utils.run_bass_kernel_spmd(
        nc, in_maps, core_ids=list(range(N_CORES)), trace=trace)
    partials = np.stack([res.results[c]["y"] for c in range(N_CORES)])
    out = partials.astype(np.float64).sum(axis=0).astype(np.float32)
    return out.reshape(B, S, D), res


def kernel(x, cos, sin, Wq, Wk, Wv, Wo):
    out, _ = run(x, cos, sin, Wq, Wk, Wv, Wo, trace=False)
    return out


# revision 10
# speedup vs baseline: 1.0878x; 1.0613x over previous
"""Trainium2 Bass kernel for nn_Attention_83743272337693.

Quantized-attention transformer block:
  q/k/v projections -> RoPE(q,k) -> per-token-per-head int8 quantization of
  q,k -> exact int8 score GEMM -> causal softmax -> attn @ v -> o_proj.

Distribution (8 NeuronCores, SPMD): tensor-parallel over heads. Core c owns
query heads 4c..4c+3 and kv head c (GQA group). Wq/Wk/Wv are sharded
column-wise, Wo row-wise; each core computes a full [S, D] partial of the
output and the host sums the 8 partials (the all-reduce).

Numerics strategy (v3):
- projections run in fp16 (full-rate PE path, 10-bit mantissa inputs); the
  int8 round() decisions flip on ~1% of elements vs the fp32 reference,
  which stays well inside the rel-err budget (simulated rel_l2 ~ 5.8e-3,
  HW-measured 6.1e-3 vs tolerance 2e-2).
- quantized q/k are small integers: exact in bf16, so the score GEMM runs
  bf16 at full rate with fp32 PSUM accumulation. k is pre-scaled by its
  dequant factor rk (bf16), q's factor rq folds into the exp scale.
- softmax avoids the row-max pass entirely: a Cauchy-Schwarz bound
  m_hat = rq*||q_int||*cummax_tile||k_s|| is used as the exp bias. probs
  live in bf16, whose exponent range absorbs the bound's overshoot
  (typical p ~ 1e-7; fp16 would denormal-underflow). scores stay in PSUM
  in 512-wide chunks (no S copy to SBUF; chunked exp with a shared bias);
  normalization 1/sum folds into the transpose diag.
- attn@v in bf16, o_proj in fp16, per-core output partial in fp16.
- rounding uses the fp32 magic-constant trick (x + 1.5*2^23 - 1.5*2^23),
  matching np.round (round-half-to-even) exactly.
"""
import numpy as np

import concourse.bass as bass
import concourse.mybir as mybir
from concourse import bacc, bass_utils
from concourse.tile import TileContext
from concourse.masks import make_causal_mask, make_identity

# Problem shape (hardcoded per contract).
B, S, D = 1, 2048, 4096
NH, NKV, HD = 32, 8, 128
N_CORES = 8
HQ = NH // N_CORES          # query heads per core (4)
ST = S // 128               # seq tiles (16)
KC = D // 128               # contraction chunks for projections (32)
SCALE = float(HD) ** -0.5
MAGIC = float(np.float32(1.5 * 2 ** 23))
MASK_VAL = -1.0e10

F32 = mybir.dt.float32
BF16 = mybir.dt.bfloat16
F16 = mybir.dt.float16
ADD = mybir.AluOpType.add
SUB = mybir.AluOpType.subtract
MUL = mybir.AluOpType.mult
MAX = mybir.AluOpType.max
Ident = mybir.ActivationFunctionType.Identity


def build():
    nc = bacc.Bacc("TRN2", target_bir_lowering=False)

    # Host-prepped layouts (see make_in_maps): per-partition-contiguous.
    xt_d = nc.dram_tensor("xt", [128, ST, KC, 128], F16, kind="ExternalInput")
    cs_d = nc.dram_tensor("cs", [128, 2, ST, HD // 2], F32, kind="ExternalInput")
    wq_d = nc.dram_tensor("wq", [128, KC, HQ * HD], F16, kind="ExternalInput")
    wkv_d = nc.dram_tensor("wkv", [128, KC, 2 * HD], F16, kind="ExternalInput")
    wo_d = nc.dram_tensor("wo", [128, HQ, D], F16, kind="ExternalInput")
    y = nc.dram_tensor("y", [S, D], F16, kind="ExternalOutput")

    with TileContext(nc) as tc:
        with (
            tc.tile_pool(name="persist", bufs=1) as persist,
            tc.tile_pool(name="small", bufs=4) as small,
        ):
            # Persistent SBUF state shared by both phases.
            qT = persist.tile([128, HQ, S], BF16, tag="qT")        # 2 MiB
            kTs = persist.tile([128, S], BF16, tag="kTs")          # 512 KiB
            v_sb = persist.tile([128, ST, HD], BF16, tag="v_sb")   # 512 KiB
            rq_sb = persist.tile([128, HQ, ST], F32, tag="rq_sb")  # exp scale
            nbias = persist.tile([128, HQ, ST], F32, tag="nbias")  # exp bias
            qnsq = persist.tile([128, HQ, ST], F32, tag="qnsq")    # ||q_int||^2
            mk2 = persist.tile([1, ST], F32, tag="mk2")            # max||k_s||^2/tile
            ident_bf = persist.tile([128, 128], BF16, tag="ident_bf")
            ident_f32 = persist.tile([128, 128], F32, tag="ident_f32")
            mask_sb = persist.tile([128, 128], F32, tag="mask_sb")
            ones_sb = persist.tile([1, 128], F32, tag="ones_sb")
            ones_col = persist.tile([128, 1], BF16, tag="ones_col")
            ones_bf = persist.tile([1, 128], BF16, tag="ones_bf")
            magic_p = persist.tile([128, 1], F32, tag="magic_p")
            magic_n = persist.tile([128, 1], F32, tag="magic_n")
            wo_sb = persist.tile([128, HQ, D], F16, tag="wo_sb")    # 4 MiB
            cs_sb = persist.tile([128, 2, ST, HD // 2], F32, tag="cs_sb")

            make_identity(nc, ident_bf[:])
            make_identity(nc, ident_f32[:])
            make_causal_mask(nc, mask_sb[:], mask_val=MASK_VAL)
            nc.gpsimd.memset(ones_sb[:], 1.0)
            nc.gpsimd.memset(ones_col[:], 1.0)
            nc.gpsimd.memset(ones_bf[:], 1.0)
            nc.gpsimd.memset(magic_p[:], MAGIC)
            nc.gpsimd.memset(magic_n[:], -MAGIC)

            # ---------------- Phase A: projections + rope + quantize ----------
            with (
                tc.tile_pool(name="wproj", bufs=1) as wpool,
                tc.tile_pool(name="xstream", bufs=3) as xpool,
                tc.tile_pool(name="ropebuf", bufs=2) as rpool,
                tc.tile_pool(name="psA", bufs=2, space="PSUM") as psA,
                tc.tile_pool(name="psT", bufs=3, space="PSUM") as psT,
            ):
                wq_sb = wpool.tile([128, KC, HQ * HD], F16, tag="wq_sb")   # 4 MiB
                wkv_sb = wpool.tile([128, KC, 2 * HD], F16, tag="wkv_sb")  # 2 MiB
                # chunked weight loads, smallest first, so the first projection
                # matmuls start as soon as their chunk lands; wo queued last.
                wsched = [(0, 1), (1, 2), (2, 4)] + [(k, k + 4) for k in range(4, KC, 4)]
                for i, (a, b) in enumerate(wsched):
                    nc.sync.dma_start(wq_sb[:, a:b, :], wq_d.ap()[:, a:b, :])
                    nc.sync.dma_start(wkv_sb[:, a:b, :], wkv_d.ap()[:, a:b, :])
                    if i == 2:
                        nc.sync.dma_start(cs_sb[:], cs_d.ap())
                nc.sync.dma_start(wo_sb[:], wo_d.ap())

                for st in range(ST):
                    xt = xpool.tile([128, KC, 128], F16, tag="xt")
                    for kq in range(0, KC, 8):
                        nc.gpsimd.dma_start(xt[:, kq:kq + 8, :],
                                            xt_d.ap()[:, st, kq:kq + 8, :])

                    ps_q = psA.tile([128, HQ * HD], F32, tag="ps_q")
                    ps_kv = psA.tile([128, 2 * HD], F32, tag="ps_kv")
                    for kc in range(KC):
                        nc.tensor.matmul(ps_q[:], xt[:, kc, :], wq_sb[:, kc, :],
                                         start=(kc == 0), stop=(kc == KC - 1))
                        nc.tensor.matmul(ps_kv[:], xt[:, kc, :], wkv_sb[:, kc, :],
                                         start=(kc == 0), stop=(kc == KC - 1))

                    # RoPE on q heads + k head (DVE, reads PSUM), batched
                    # across heads via broadcast APs on cos/sin.
                    half = HD // 2
                    cos_t = cs_sb[:, 0, st, :]
                    sin_t = cs_sb[:, 1, st, :]
                    cos_q = cos_t.unsqueeze(1).broadcast_to([128, HQ, half])
                    sin_q = sin_t.unsqueeze(1).broadcast_to([128, HQ, half])
                    # rope layout: [128, HQ+1, HD] (heads 0..3 = q, 4 = k)
                    rope = rpool.tile([128, HQ + 1, HD], F32, tag="rope")
                    tmp = rpool.tile([128, HQ + 1, half], F32, tag="tmp")
                    q_r = ps_q[:].rearrange("p (h t f) -> p h t f", h=HQ, t=2)
                    k_r = ps_kv[:, 0:HD].rearrange("p (t f) -> p t f", t=2)
                    # q heads (batched over h)
                    nc.vector.tensor_tensor(rope[:, :HQ, 0:half], q_r[:, :, 0, :], cos_q, op=MUL)
                    nc.vector.tensor_tensor(tmp[:, :HQ, :], q_r[:, :, 1, :], sin_q, op=MUL)
                    nc.vector.tensor_tensor(rope[:, :HQ, 0:half], rope[:, :HQ, 0:half],
                                            tmp[:, :HQ, :], op=SUB)
                    nc.vector.tensor_tensor(rope[:, :HQ, half:HD], q_r[:, :, 0, :], sin_q, op=MUL)
                    nc.vector.tensor_tensor(tmp[:, :HQ, :], q_r[:, :, 1, :], cos_q, op=MUL)
                    nc.vector.tensor_tensor(rope[:, :HQ, half:HD], rope[:, :HQ, half:HD],
                                            tmp[:, :HQ, :], op=ADD)
                    # k head
                    nc.vector.tensor_tensor(rope[:, HQ, 0:half], k_r[:, 0, :], cos_t, op=MUL)
                    nc.vector.tensor_tensor(tmp[:, HQ, :], k_r[:, 1, :], sin_t, op=MUL)
                    nc.vector.tensor_tensor(rope[:, HQ, 0:half], rope[:, HQ, 0:half],
                                            tmp[:, HQ, :], op=SUB)
                    nc.vector.tensor_tensor(rope[:, HQ, half:HD], k_r[:, 0, :], sin_t, op=MUL)
                    nc.vector.tensor_tensor(tmp[:, HQ, :], k_r[:, 1, :], cos_t, op=MUL)
                    nc.vector.tensor_tensor(rope[:, HQ, half:HD], rope[:, HQ, half:HD],
                                            tmp[:, HQ, :], op=ADD)

                    # v: straight cast to bf16 (no rope/quant).
                    nc.scalar.copy(v_sb[:, st, :], ps_kv[:, HD:2 * HD])

                    # Quantize all 5 rope'd heads at once:
                    # am = absmax, s = 127/clip(am,1e-5), int = round(x*s).
                    am = small.tile([128, HQ + 1], F32, tag="am")
                    nc.vector.tensor_reduce(am[:], rope[:], axis=mybir.AxisListType.X,
                                            op=MAX, apply_absolute_value=True)
                    nc.vector.tensor_scalar_max(am[:], am[:], 1e-5)
                    # dequant factors: rq = am*scale/127 (q heads), rk = am/127
                    nc.vector.tensor_scalar_mul(rq_sb[:, :, st], am[:, 0:HQ],
                                                SCALE / 127.0)
                    rk_col = small.tile([128, 1], BF16, tag="rk_col")
                    nc.vector.tensor_scalar_mul(rk_col[:], am[:, HQ:HQ + 1],
                                                1.0 / 127.0)
                    sc = small.tile([128, HQ + 1], F32, tag="sc")
                    nc.vector.reciprocal(sc[:], am[:])
                    nc.vector.tensor_scalar_mul(sc[:], sc[:], 127.0)
                    rnd = rpool.tile([128, HQ + 1, HD], F32, tag="rnd")
                    sc_b = sc[:].unsqueeze(2).broadcast_to([128, HQ + 1, HD])
                    nc.gpsimd.tensor_tensor(rnd[:], rope[:], sc_b, op=MUL)
                    rnd_f = rnd[:].rearrange("p h f -> p (h f)")
                    # round via magic constant; the two adds run on ACT
                    nc.scalar.activation(rnd_f, rnd_f, Ident, bias=magic_p[:])
                    qint = rpool.tile([128, (HQ + 1) * HD], BF16, tag="qint")
                    nc.scalar.activation(qint[:], rnd_f, Ident, bias=magic_n[:])

                    # ||q_int||^2 per q head (for the softmax bound)
                    qint_r = qint[:].rearrange("p (h f) -> p h f", h=HQ + 1)
                    sq = rpool.tile([128, HQ, HD], F32, tag="sq")
                    nc.gpsimd.tensor_tensor(sq[:], qint_r[:, 0:HQ, :],
                                            qint_r[:, 0:HQ, :], op=MUL)
                    nc.vector.tensor_reduce(qnsq[:, :, st], sq[:],
                                            axis=mybir.AxisListType.X, op=ADD)

                    # rk broadcast for this tile: [128,1] -> [1,128] -> [128,128]
                    ps_r1 = psT.tile([1, 128], F32, tag="ps_t")
                    nc.tensor.matmul(ps_r1[:], rk_col[:], ident_bf[:])
                    rk1 = small.tile([1, 128], BF16, tag="rk1")
                    nc.vector.tensor_copy(rk1[:], ps_r1[:])
                    ps_rb = psT.tile([128, 128], F32, tag="ps_t")
                    nc.tensor.matmul(ps_rb[:], ones_bf[:], rk1[:])
                    rkb = rpool.tile([128, 128], F32, tag="rkb")
                    nc.vector.tensor_copy(rkb[:], ps_rb[:])

                    # PE transposes of the quantized heads into [hd, seq] layout.
                    sb = slice(st * 128, (st + 1) * 128)
                    for hh in range(HQ + 1):
                        o = hh * HD
                        ps_t = psT.tile([128, 128], F32, tag="ps_t")
                        nc.tensor.matmul(ps_t[:], qint[:, o:o + HD], ident_bf[:])
                        if hh < HQ:
                            nc.scalar.copy(qT[:, hh, sb], ps_t[:])
                        else:
                            # scale k^T columns by rk while copying out
                            nc.vector.tensor_tensor(kTs[:, sb], ps_t[:], rkb[:],
                                                    op=MUL)
                    # max column norm^2 of this kTs block (softmax bound)
                    ksq = rpool.tile([128, 128], BF16, tag="ksq")
                    nc.vector.tensor_tensor(ksq[:], kTs[:, sb], kTs[:, sb], op=MUL)
                    ps_n = psT.tile([1, 128], F32, tag="ps_t")
                    nc.tensor.matmul(ps_n[:], ones_col[:], ksq[:])
                    nc.vector.tensor_reduce(mk2[:, st:st + 1], ps_n[:],
                                            axis=mybir.AxisListType.X, op=MAX)

            # ---------------- softmax bound bias (tiny) -----------------------
            with (
                tc.tile_pool(name="rkb2", bufs=1) as rkpool,
                tc.tile_pool(name="psR", bufs=2, space="PSUM") as psR,
            ):
                # cummax over tiles, sqrt, broadcast to all partitions
                mk2r = rkpool.tile([1, ST], F32, tag="mk2r")
                nc.vector.tensor_tensor_scan(
                    mk2r[:], mk2[:], mk2[:], initial=0.0, op0=MAX, op1=MAX)
                mkr = rkpool.tile([1, ST], F32, tag="mkr")
                nc.scalar.sqrt(mkr[:], mk2r[:])
                ps_mk = psR.tile([128, ST], F32, tag="ps_mk")
                nc.tensor.matmul(ps_mk[:], ones_sb[:], mkr[:])
                mk_b = rkpool.tile([128, ST], F32, tag="mk_b")
                nc.vector.tensor_copy(mk_b[:], ps_mk[:])
                # nbias = -(rq*||q||) * Mk[qt]
                qn = rkpool.tile([128, HQ, ST], F32, tag="qn")
                nc.scalar.sqrt(qn[:], qnsq[:])
                rqn = rkpool.tile([128, HQ, ST], F32, tag="rqn")
                nc.vector.tensor_tensor(rqn[:], rq_sb[:], qn[:], op=MUL)
                mk_q = mk_b[:].unsqueeze(1).broadcast_to([128, HQ, ST])
                nc.vector.scalar_tensor_tensor(
                    nbias[:], rqn[:], -1.0, mk_q, op0=MUL, op1=MUL)

            # ---------------- Phase B: attention + o_proj -----------------
            with (
                tc.tile_pool(name="sbp", bufs=5) as sbp,
                tc.tile_pool(name="pbuf", bufs=2) as pbuf,
                tc.tile_pool(name="obuf", bufs=3) as obuf,
                tc.tile_pool(name="psS", bufs=3, space="PSUM") as psS,
                tc.tile_pool(name="psP", bufs=2, space="PSUM") as psP,
                tc.tile_pool(name="psV", bufs=1, space="PSUM") as psV,
                tc.tile_pool(name="psO", bufs=2, space="PSUM") as psO,
            ):
                for qt in range(ST):
                    K = (qt + 1) * 128
                    nchunk = (K + 511) // 512
                    pT = pbuf.tile([128, HQ, S], BF16, tag="pT")
                    heads = []
                    for h in range(HQ):
                        lhs_q = qT[:, h, qt * 128:(qt + 1) * 128]
                        p_sb = sbp.tile([128, S], BF16, tag="p_sb")
                        ssum = small.tile([128, 4], F32, tag="ssum")
                        rq_h = rq_sb[:, h, qt:qt + 1]
                        nb_h = nbias[:, h, qt:qt + 1]
                        # 512-wide PSUM-resident chunks, shared global bias
                        for ci in range(nchunk):
                            c0 = ci * 512
                            cw = min(K, c0 + 512) - c0
                            ps_S = psS.tile([128, 512], F32, tag="ps_S")
                            nc.tensor.matmul(ps_S[:, 0:cw], lhs_q,
                                             kTs[:, c0:c0 + cw])
                            if ci == nchunk - 1:
                                # causal mask on the diagonal block
                                db = qt * 128 - c0
                                nc.vector.tensor_tensor(
                                    ps_S[:, db:db + 128], ps_S[:, db:db + 128],
                                    mask_sb[:], op=ADD)
                            nc.scalar.activation(
                                p_sb[:, c0:c0 + cw], ps_S[:, 0:cw],
                                mybir.ActivationFunctionType.Exp,
                                bias=nb_h, scale=rq_h,
                                accum_out=ssum[:, ci:ci + 1])
                        w = small.tile([128, 1], F32, tag="w")
                        if nchunk > 1:
                            tot = small.tile([128, 1], F32, tag="tot")
                            nc.vector.tensor_reduce(tot[:], ssum[:, 0:nchunk],
                                                    axis=mybir.AxisListType.X,
                                                    op=ADD)
                            nc.vector.reciprocal(w[:], tot[:])
                        else:
                            nc.vector.reciprocal(w[:], ssum[:, 0:1])
                        diag = sbp.tile([128, 128], BF16, tag="diag")
                        nc.vector.tensor_scalar_mul(diag[:], ident_bf[:], w[:])
                        heads.append((p_sb, diag))
                    # p^T (scaled by 1/sum) via PE, two blocks per bank;
                    # separate stream so score matmuls don't thrash stationary
                    for h in range(HQ):
                        p_sb, diag = heads[h]
                        for kc in range(0, qt + 1, 2):
                            kn = min(2, qt + 1 - kc)
                            ps_p = psP.tile([128, 256], F32, tag="ps_p")
                            for j in range(kn):
                                nc.tensor.matmul(
                                    ps_p[:, j * 128:(j + 1) * 128],
                                    p_sb[:, (kc + j) * 128:(kc + j + 1) * 128],
                                    diag[:])
                            if kc % 4 < 2:
                                nc.vector.tensor_copy(
                                    pT[:, h, kc * 128:(kc + kn) * 128],
                                    ps_p[:, :kn * 128])
                            else:
                                nc.scalar.copy(
                                    pT[:, h, kc * 128:(kc + kn) * 128],
                                    ps_p[:, :kn * 128])
                    # attn @ v for all 4 heads at once (N=512 moving)
                    ps_oh = psV.tile([128, HQ * 128], F32, tag="ps_oh")
                    for kc in range(qt + 1):
                        nc.tensor.matmul(
                            ps_oh[:], v_sb[:, kc, :],
                            pT[:, :, kc * 128:(kc + 1) * 128],
                            start=(kc == 0), stop=(kc == qt))
                    ohT = pbuf.tile([128, HQ * 128], F16, tag="ohT")
                    nc.scalar.copy(ohT[:], ps_oh[:])
                    # o_proj for this q-tile: accumulate the 4 heads.
                    for b in range(D // 512):
                        ps_O = psO.tile([128, 512], F32, tag="ps_O")
                        for h in range(HQ):
                            nc.tensor.matmul(
                                ps_O[:], ohT[:, h * 128:(h + 1) * 128],
                                wo_sb[:, h, b * 512:(b + 1) * 512],
                                start=(h == 0), stop=(h == HQ - 1))
                        out_t = obuf.tile([128, 512], F16, tag="out_t")
                        if b % 2 == 0:
                            nc.vector.tensor_copy(out_t[:], ps_O[:])
                        else:
                            nc.scalar.copy(out_t[:], ps_O[:])
                        nc.gpsimd.dma_start(
                            y.ap()[qt * 128:(qt + 1) * 128,
                                   b * 512:(b + 1) * 512], out_t[:])

    nc.finalize()
    return nc


_NC_CACHE = None


def _get_nc():
    global _NC_CACHE
    if _NC_CACHE is None:
        _NC_CACHE = build()
    return _NC_CACHE


def make_in_maps(x, cos, sin, Wq, Wk, Wv, Wo):
    """Shard the full inputs into the 8 per-core input maps."""
    x = np.asarray(x, np.float32).reshape(S, D)
    # xt[p, st, kc, s] = x[st*128+s, kc*128+p]  (fp16)
    xt = np.ascontiguousarray(
        x.reshape(ST, 128, KC, 128).transpose(3, 0, 2, 1)).astype(np.float16)
    # cs[p, {cos,sin}, st, f] = {cos,sin}[st*128+p, f]
    cosr = np.asarray(cos, np.float32).reshape(ST, 128, HD // 2)
    sinr = np.asarray(sin, np.float32).reshape(ST, 128, HD // 2)
    cs = np.ascontiguousarray(
        np.stack([cosr, sinr], axis=0).transpose(2, 0, 1, 3))
    Wq = np.asarray(Wq, np.float32)
    Wk = np.asarray(Wk, np.float32)
    Wv = np.asarray(Wv, np.float32)
    Wo = np.asarray(Wo, np.float32)
    in_maps = []
    for c in range(N_CORES):
        qs = slice(c * HQ * HD, (c + 1) * HQ * HD)
        ks = slice(c * HD, (c + 1) * HD)
        # wq[p, kc, n] = Wq[kc*128+p, qs][n]
        wq = np.ascontiguousarray(
            Wq[:, qs].reshape(KC, 128, HQ * HD).transpose(1, 0, 2)
        ).astype(np.float16)
        wkv_full = np.concatenate([Wk[:, ks], Wv[:, ks]], axis=1)
        wkv = np.ascontiguousarray(
            wkv_full.reshape(KC, 128, 2 * HD).transpose(1, 0, 2)
        ).astype(np.float16)
        # wo[p, h, n] = Wo[qs][h*128+p, n]
        wo = np.ascontiguousarray(
            Wo[qs, :].reshape(HQ, 128, D).transpose(1, 0, 2)
        ).astype(np.float16)
        in_maps.append({"xt": xt, "cs": cs, "wq": wq, "wkv": wkv, "wo": wo})
    return in_maps


def run(x, cos, sin, Wq, Wk, Wv, Wo, trace=False):
    nc = _get_nc()
    in_maps = make_in_maps(x, cos, sin, Wq, Wk, Wv, Wo)
    res = bass_utils.run_bass_kernel_spmd(
        nc, in_maps, core_ids=list(range(N_CORES)), trace=trace)
    partials = np.stack([res.results[c]["y"] for c in range(N_CORES)])
    out = partials.astype(np.float64).sum(axis=0).astype(np.float32)
    return out.reshape(B, S, D), res


def kernel(x, cos, sin, Wq, Wk, Wv, Wo):
    out, _ = run(x, cos, sin, Wq, Wk, Wv, Wo, trace=False)
    return out


# revision 12
# speedup vs baseline: 1.0967x; 1.0082x over previous
"""Trainium2 Bass kernel for nn_Attention_83743272337693.

Quantized-attention transformer block:
  q/k/v projections -> RoPE(q,k) -> per-token-per-head int8 quantization of
  q,k -> exact int8 score GEMM -> causal softmax -> attn @ v -> o_proj.

Distribution (8 NeuronCores, SPMD): tensor-parallel over heads. Core c owns
query heads 4c..4c+3 and kv head c (GQA group). Wq/Wk/Wv are sharded
column-wise, Wo row-wise; each core computes a full [S, D] partial of the
output and the host sums the 8 partials (the all-reduce).

Numerics strategy (v3):
- projections run in fp16 (full-rate PE path, 10-bit mantissa inputs); the
  int8 round() decisions flip on ~1% of elements vs the fp32 reference,
  which stays well inside the rel-err budget (simulated rel_l2 ~ 5.8e-3,
  HW-measured 6.1e-3 vs tolerance 2e-2).
- quantized q/k are small integers: exact in bf16, so the score GEMM runs
  bf16 at full rate with fp32 PSUM accumulation. k is pre-scaled by its
  dequant factor rk (bf16), q's factor rq folds into the exp scale.
- softmax avoids the row-max pass entirely: a Cauchy-Schwarz bound
  m_hat = rq*||q_int||*cummax_tile||k_s|| is used as the exp bias. probs
  live in bf16, whose exponent range absorbs the bound's overshoot
  (typical p ~ 1e-7; fp16 would denormal-underflow). scores stay in PSUM
  in 512-wide chunks (no S copy to SBUF; chunked exp with a shared bias);
  normalization 1/sum folds into the transpose diag.
- attn@v in bf16, o_proj in fp16, per-core output partial in fp16.
- rounding uses the fp32 magic-constant trick (x + 1.5*2^23 - 1.5*2^23),
  matching np.round (round-half-to-even) exactly.
"""
import numpy as np

import concourse.bass as bass
import concourse.mybir as mybir
from concourse import bacc, bass_utils
from concourse.tile import TileContext
from concourse.masks import make_causal_mask, make_identity

# Problem shape (hardcoded per contract).
B, S, D = 1, 2048, 4096
NH, NKV, HD = 32, 8, 128
N_CORES = 8
HQ = NH // N_CORES          # query heads per core (4)
ST = S // 128               # seq tiles (16)
KC = D // 128               # contraction chunks for projections (32)
SCALE = float(HD) ** -0.5
MAGIC = float(np.float32(1.5 * 2 ** 23))
MASK_VAL = -1.0e10

F32 = mybir.dt.float32
BF16 = mybir.dt.bfloat16
F16 = mybir.dt.float16
ADD = mybir.AluOpType.add
SUB = mybir.AluOpType.subtract
MUL = mybir.AluOpType.mult
MAX = mybir.AluOpType.max
Ident = mybir.ActivationFunctionType.Identity


def build():
    nc = bacc.Bacc("TRN2", target_bir_lowering=False)

    # Host-prepped layouts (see make_in_maps): per-partition-contiguous.
    xt_d = nc.dram_tensor("xt", [128, ST, KC, 128], F16, kind="ExternalInput")
    cs_d = nc.dram_tensor("cs", [128, 2, ST, HD // 2], F32, kind="ExternalInput")
    wq_d = nc.dram_tensor("wq", [128, KC, HQ * HD], F16, kind="ExternalInput")
    wkv_d = nc.dram_tensor("wkv", [128, KC, 2 * HD], F16, kind="ExternalInput")
    wo_d = nc.dram_tensor("wo", [128, HQ, D], F16, kind="ExternalInput")
    y = nc.dram_tensor("y", [S, D], F16, kind="ExternalOutput")

    with TileContext(nc) as tc:
        with (
            tc.tile_pool(name="persist", bufs=1) as persist,
            tc.tile_pool(name="small", bufs=4) as small,
        ):
            # Persistent SBUF state shared by both phases.
            qT = persist.tile([128, HQ, S], BF16, tag="qT")        # 2 MiB
            kTs = persist.tile([128, S], BF16, tag="kTs")          # 512 KiB
            v_sb = persist.tile([128, ST, HD], BF16, tag="v_sb")   # 512 KiB
            rq_sb = persist.tile([128, HQ, ST], F32, tag="rq_sb")  # exp scale
            nbias = persist.tile([128, HQ, ST], F32, tag="nbias")  # exp bias
            qnsq = persist.tile([128, HQ, ST], F32, tag="qnsq")    # ||q_int||^2
            mk2 = persist.tile([1, ST], F32, tag="mk2")            # max||k_s||^2/tile
            mkrun = persist.tile([1, 1], F32, tag="mkrun")         # running max
            ident_bf = persist.tile([128, 128], BF16, tag="ident_bf")
            ident_f32 = persist.tile([128, 128], F32, tag="ident_f32")
            mask_sb = persist.tile([128, 128], F32, tag="mask_sb")
            ones_sb = persist.tile([1, 128], F32, tag="ones_sb")
            ones_col = persist.tile([128, 1], BF16, tag="ones_col")
            ones_bf = persist.tile([1, 128], BF16, tag="ones_bf")
            magic_p = persist.tile([128, 1], F32, tag="magic_p")
            magic_n = persist.tile([128, 1], F32, tag="magic_n")
            wo_sb = persist.tile([128, HQ, D], F16, tag="wo_sb")    # 4 MiB
            cs_sb = persist.tile([128, 2, ST, HD // 2], F32, tag="cs_sb")

            make_identity(nc, ident_bf[:])
            make_identity(nc, ident_f32[:])
            make_causal_mask(nc, mask_sb[:], mask_val=MASK_VAL)
            nc.gpsimd.memset(ones_sb[:], 1.0)
            nc.gpsimd.memset(ones_col[:], 1.0)
            nc.gpsimd.memset(ones_bf[:], 1.0)
            nc.gpsimd.memset(magic_p[:], MAGIC)
            nc.gpsimd.memset(magic_n[:], -MAGIC)

            # ---------------- Phase A: projections + rope + quantize ----------
            with (
                tc.tile_pool(name="wproj", bufs=1) as wpool,
                tc.tile_pool(name="xstream", bufs=3) as xpool,
                tc.tile_pool(name="ropebuf", bufs=2) as rpool,
                tc.tile_pool(name="psA", bufs=2, space="PSUM") as psA,
                tc.tile_pool(name="psT", bufs=3, space="PSUM") as psT,
            ):
                wq_sb = wpool.tile([128, KC, HQ * HD], F16, tag="wq_sb")   # 4 MiB
                wkv_sb = wpool.tile([128, KC, 2 * HD], F16, tag="wkv_sb")  # 2 MiB
                # chunked weight loads, smallest first, so the first projection
                # matmuls start as soon as their chunk lands; wo queued last.
                wsched = ([(k, k + 1) for k in range(4)] + [(k, k + 2) for k in range(4, 16, 2)]
                          + [(k, k + 4) for k in range(16, KC, 4)])
                for i, (a, b) in enumerate(wsched):
                    nc.sync.dma_start(wq_sb[:, a:b, :], wq_d.ap()[:, a:b, :])
                    nc.sync.dma_start(wkv_sb[:, a:b, :], wkv_d.ap()[:, a:b, :])
                    if i == 2:
                        nc.sync.dma_start(cs_sb[:], cs_d.ap())
                nc.sync.dma_start(wo_sb[:], wo_d.ap())

                for st in range(ST):
                    xt = xpool.tile([128, KC, 128], F16, tag="xt")
                    for kq in range(0, KC, 8):
                        nc.gpsimd.dma_start(xt[:, kq:kq + 8, :],
                                            xt_d.ap()[:, st, kq:kq + 8, :])

                    ps_q = psA.tile([128, HQ * HD], F32, tag="ps_q")
                    ps_kv = psA.tile([128, 2 * HD], F32, tag="ps_kv")
                    for kc in range(KC):
                        nc.tensor.matmul(ps_q[:], xt[:, kc, :], wq_sb[:, kc, :],
                                         start=(kc == 0), stop=(kc == KC - 1))
                        nc.tensor.matmul(ps_kv[:], xt[:, kc, :], wkv_sb[:, kc, :],
                                         start=(kc == 0), stop=(kc == KC - 1))

                    # RoPE on q heads + k head (DVE, reads PSUM), batched
                    # across heads via broadcast APs on cos/sin.
                    half = HD // 2
                    cos_t = cs_sb[:, 0, st, :]
                    sin_t = cs_sb[:, 1, st, :]
                    cos_q = cos_t.unsqueeze(1).broadcast_to([128, HQ, half])
                    sin_q = sin_t.unsqueeze(1).broadcast_to([128, HQ, half])
                    # rope layout: [128, HQ+1, HD] (heads 0..3 = q, 4 = k)
                    rope = rpool.tile([128, HQ + 1, HD], F32, tag="rope")
                    tmp = rpool.tile([128, HQ + 1, half], F32, tag="tmp")
                    q_r = ps_q[:].rearrange("p (h t f) -> p h t f", h=HQ, t=2)
                    k_r = ps_kv[:, 0:HD].rearrange("p (t f) -> p t f", t=2)
                    # q heads (batched over h)
                    nc.vector.tensor_tensor(rope[:, :HQ, 0:half], q_r[:, :, 0, :], cos_q, op=MUL)
                    nc.vector.tensor_tensor(tmp[:, :HQ, :], q_r[:, :, 1, :], sin_q, op=MUL)
                    nc.vector.tensor_tensor(rope[:, :HQ, 0:half], rope[:, :HQ, 0:half],
                                            tmp[:, :HQ, :], op=SUB)
                    nc.vector.tensor_tensor(rope[:, :HQ, half:HD], q_r[:, :, 0, :], sin_q, op=MUL)
                    nc.vector.tensor_tensor(tmp[:, :HQ, :], q_r[:, :, 1, :], cos_q, op=MUL)
                    nc.vector.tensor_tensor(rope[:, :HQ, half:HD], rope[:, :HQ, half:HD],
                                            tmp[:, :HQ, :], op=ADD)
                    # k head
                    nc.vector.tensor_tensor(rope[:, HQ, 0:half], k_r[:, 0, :], cos_t, op=MUL)
                    nc.vector.tensor_tensor(tmp[:, HQ, :], k_r[:, 1, :], sin_t, op=MUL)
                    nc.vector.tensor_tensor(rope[:, HQ, 0:half], rope[:, HQ, 0:half],
                                            tmp[:, HQ, :], op=SUB)
                    nc.vector.tensor_tensor(rope[:, HQ, half:HD], k_r[:, 0, :], sin_t, op=MUL)
                    nc.vector.tensor_tensor(tmp[:, HQ, :], k_r[:, 1, :], cos_t, op=MUL)
                    nc.vector.tensor_tensor(rope[:, HQ, half:HD], rope[:, HQ, half:HD],
                                            tmp[:, HQ, :], op=ADD)

                    # v: straight cast to bf16 (no rope/quant).
                    nc.scalar.copy(v_sb[:, st, :], ps_kv[:, HD:2 * HD])

                    # Quantize all 5 rope'd heads at once:
                    # am = absmax, s = 127/clip(am,1e-5), int = round(x*s).
                    am = small.tile([128, HQ + 1], F32, tag="am")
                    nc.vector.tensor_reduce(am[:], rope[:], axis=mybir.AxisListType.X,
                                            op=MAX, apply_absolute_value=True)
                    nc.vector.tensor_scalar_max(am[:], am[:], 1e-5)
                    # dequant factors: rq = am*scale/127 (q heads), rk = am/127
                    nc.vector.tensor_scalar_mul(rq_sb[:, :, st], am[:, 0:HQ],
                                                SCALE / 127.0)
                    rk_col = small.tile([128, 1], F32, tag="rk_col")
                    nc.vector.tensor_scalar_mul(rk_col[:], am[:, HQ:HQ + 1],
                                                1.0 / 127.0)
                    sc = small.tile([128, HQ + 1], F32, tag="sc")
                    nc.vector.reciprocal(sc[:], am[:])
                    nc.vector.tensor_scalar_mul(sc[:], sc[:], 127.0)
                    rnd = rpool.tile([128, HQ + 1, HD], F32, tag="rnd")
                    sc_b = sc[:].unsqueeze(2).broadcast_to([128, HQ + 1, HD])
                    nc.gpsimd.tensor_tensor(rnd[:], rope[:], sc_b, op=MUL)
                    rnd_f = rnd[:].rearrange("p h f -> p (h f)")
                    # round via magic constant; the two adds run on ACT
                    nc.scalar.activation(rnd_f, rnd_f, Ident, bias=magic_p[:])
                    qint = rpool.tile([128, (HQ + 1) * HD], BF16, tag="qint")
                    nc.scalar.activation(qint[:], rnd_f, Ident, bias=magic_n[:])

                    # ||q_int||^2 per q head (for the softmax bound)
                    qint_r = qint[:].rearrange("p (h f) -> p h f", h=HQ + 1)
                    sq = rpool.tile([128, HQ, HD], F32, tag="sq")
                    nc.gpsimd.tensor_tensor(sq[:], qint_r[:, 0:HQ, :],
                                            qint_r[:, 0:HQ, :], op=MUL)
                    nc.vector.tensor_reduce(qnsq[:, :, st], sq[:],
                                            axis=mybir.AxisListType.X, op=ADD)

                    # rk broadcast for this tile: [128,1] -> [1,128] -> [128,128]
                    ps_r1 = psT.tile([1, 128], F32, tag="ps_t")
                    nc.tensor.matmul(ps_r1[:], rk_col[:], ident_f32[:])
                    rk1 = small.tile([1, 128], F32, tag="rk1")
                    nc.vector.tensor_copy(rk1[:], ps_r1[:])
                    ps_rb = psT.tile([128, 128], F32, tag="ps_t")
                    nc.tensor.matmul(ps_rb[:], ones_sb[:], rk1[:])
                    rkb = rpool.tile([128, 128], F32, tag="rkb")
                    nc.vector.tensor_copy(rkb[:], ps_rb[:])

                    # PE transposes of the quantized heads into [hd, seq] layout.
                    sb = slice(st * 128, (st + 1) * 128)
                    for hh in range(HQ + 1):
                        o = hh * HD
                        ps_t = psT.tile([128, 128], F32, tag="ps_t")
                        nc.tensor.matmul(ps_t[:], qint[:, o:o + HD], ident_bf[:])
                        if hh < HQ:
                            nc.scalar.copy(qT[:, hh, sb], ps_t[:])
                        else:
                            # scale k^T columns by rk while copying out
                            nc.vector.tensor_tensor(kTs[:, sb], ps_t[:], rkb[:],
                                                    op=MUL)
                    # max column norm^2 of this kTs block (softmax bound)
                    ksq = rpool.tile([128, 128], BF16, tag="ksq")
                    nc.vector.tensor_tensor(ksq[:], kTs[:, sb], kTs[:, sb], op=MUL)
                    ps_n = psT.tile([1, 128], F32, tag="ps_t")
                    nc.tensor.matmul(ps_n[:], ones_col[:], ksq[:])
                    nc.vector.tensor_reduce(mk2[:, st:st + 1], ps_n[:],
                                            axis=mybir.AxisListType.X, op=MAX)
                    # softmax bound for this row tile:
                    # nbias[:,h,st] = -rq*||q_int||*sqrt(cummax mk2)
                    if st == 0:
                        nc.vector.tensor_copy(mkrun[:], mk2[:, 0:1])
                    else:
                        nc.vector.tensor_tensor(mkrun[:], mkrun[:],
                                                mk2[:, st:st + 1], op=MAX)
                    mks = small.tile([1, 1], F32, tag="mks")
                    nc.scalar.sqrt(mks[:], mkrun[:])
                    ps_m1 = psT.tile([128, 1], F32, tag="ps_t")
                    nc.tensor.matmul(ps_m1[:], ones_sb[:], mks[:])
                    mkb1 = small.tile([128, 1], F32, tag="mkb1")
                    nc.vector.tensor_copy(mkb1[:], ps_m1[:])
                    qn_t = small.tile([128, HQ], F32, tag="qn_t")
                    nc.scalar.sqrt(qn_t[:], qnsq[:, :, st])
                    rqn_t = small.tile([128, HQ], F32, tag="rqn_t")
                    nc.vector.tensor_tensor(rqn_t[:], rq_sb[:, :, st], qn_t[:],
                                            op=MUL)
                    nc.vector.scalar_tensor_tensor(
                        nbias[:, :, st], rqn_t[:], -1.0,
                        mkb1[:].broadcast_to([128, HQ]), op0=MUL, op1=MUL)

            # ---------------- Phase B: attention + o_proj -----------------
            with (
                tc.tile_pool(name="sbp", bufs=5) as sbp,
                tc.tile_pool(name="pbuf", bufs=2) as pbuf,
                tc.tile_pool(name="obuf", bufs=3) as obuf,
                tc.tile_pool(name="psS", bufs=3, space="PSUM") as psS,
                tc.tile_pool(name="psP", bufs=2, space="PSUM") as psP,
                tc.tile_pool(name="psV", bufs=1, space="PSUM") as psV,
                tc.tile_pool(name="psO", bufs=2, space="PSUM") as psO,
            ):
                for qt in range(ST):
                    K = (qt + 1) * 128
                    nchunk = (K + 511) // 512
                    pT = pbuf.tile([128, HQ, S], BF16, tag="pT")
                    heads = []
                    for h in range(HQ):
                        lhs_q = qT[:, h, qt * 128:(qt + 1) * 128]
                        p_sb = sbp.tile([128, S], BF16, tag="p_sb")
                        ssum = small.tile([128, 4], F32, tag="ssum")
                        rq_h = rq_sb[:, h, qt:qt + 1]
                        nb_h = nbias[:, h, qt:qt + 1]
                        # 512-wide PSUM-resident chunks, shared global bias
                        for ci in range(nchunk):
                            c0 = ci * 512
                            cw = min(K, c0 + 512) - c0
                            ps_S = psS.tile([128, 512], F32, tag="ps_S")
                            nc.tensor.matmul(ps_S[:, 0:cw], lhs_q,
                                             kTs[:, c0:c0 + cw])
                            if ci == nchunk - 1:
                                # causal mask on the diagonal block
                                db = qt * 128 - c0
                                nc.vector.tensor_tensor(
                                    ps_S[:, db:db + 128], ps_S[:, db:db + 128],
                                    mask_sb[:], op=ADD)
                            nc.scalar.activation(
                                p_sb[:, c0:c0 + cw], ps_S[:, 0:cw],
                                mybir.ActivationFunctionType.Exp,
                                bias=nb_h, scale=rq_h,
                                accum_out=ssum[:, ci:ci + 1])
                        w = small.tile([128, 1], F32, tag="w")
                        if nchunk > 1:
                            tot = small.tile([128, 1], F32, tag="tot")
                            nc.vector.tensor_reduce(tot[:], ssum[:, 0:nchunk],
                                                    axis=mybir.AxisListType.X,
                                                    op=ADD)
                            nc.vector.reciprocal(w[:], tot[:])
                        else:
                            nc.vector.reciprocal(w[:], ssum[:, 0:1])
                        diag = sbp.tile([128, 128], BF16, tag="diag")
                        nc.vector.tensor_scalar_mul(diag[:], ident_bf[:], w[:])
                        heads.append((p_sb, diag))
                    # p^T (scaled by 1/sum) via PE, two blocks per bank;
                    # separate stream so score matmuls don't thrash stationary
                    for h in range(HQ):
                        p_sb, diag = heads[h]
                        for kc in range(0, qt + 1, 2):
                            kn = min(2, qt + 1 - kc)
                            ps_p = psP.tile([128, 256], F32, tag="ps_p")
                            for j in range(kn):
                                nc.tensor.matmul(
                                    ps_p[:, j * 128:(j + 1) * 128],
                                    p_sb[:, (kc + j) * 128:(kc + j + 1) * 128],
                                    diag[:])
                            if kc % 4 < 2:
                                nc.vector.tensor_copy(
                                    pT[:, h, kc * 128:(kc + kn) * 128],
                                    ps_p[:, :kn * 128])
                            else:
                                nc.scalar.copy(
                                    pT[:, h, kc * 128:(kc + kn) * 128],
                                    ps_p[:, :kn * 128])
                    # attn @ v for all 4 heads at once (N=512 moving)
                    ps_oh = psV.tile([128, HQ * 128], F32, tag="ps_oh")
                    for kc in range(qt + 1):
                        nc.tensor.matmul(
                            ps_oh[:], v_sb[:, kc, :],
                            pT[:, :, kc * 128:(kc + 1) * 128],
                            start=(kc == 0), stop=(kc == qt))
                    ohT = pbuf.tile([128, HQ * 128], F16, tag="ohT")
                    for h in range(HQ):
                        hb = slice(h * 128, (h + 1) * 128)
                        if h % 2 == 0:
                            nc.scalar.copy(ohT[:, hb], ps_oh[:, hb])
                        else:
                            nc.vector.tensor_copy(ohT[:, hb], ps_oh[:, hb])
                    # o_proj for this q-tile: accumulate the 4 heads.
                    for b in range(D // 512):
                        ps_O = psO.tile([128, 512], F32, tag="ps_O")
                        for h in range(HQ):
                            nc.tensor.matmul(
                                ps_O[:], ohT[:, h * 128:(h + 1) * 128],
                                wo_sb[:, h, b * 512:(b + 1) * 512],
                                start=(h == 0), stop=(h == HQ - 1))
                        out_t = obuf.tile([128, 512], F16, tag="out_t")
                        if b % 2 == 0:
                            nc.vector.tensor_copy(out_t[:], ps_O[:])
                        else:
                            nc.scalar.copy(out_t[:], ps_O[:])
                        nc.gpsimd.dma_start(
                            y.ap()[qt * 128:(qt + 1) * 128,
                                   b * 512:(b + 1) * 512], out_t[:])

    nc.finalize()
    return nc


_NC_CACHE = None


def _get_nc():
    global _NC_CACHE
    if _NC_CACHE is None:
        _NC_CACHE = build()
    return _NC_CACHE


def make_in_maps(x, cos, sin, Wq, Wk, Wv, Wo):
    """Shard the full inputs into the 8 per-core input maps."""
    x = np.asarray(x, np.float32).reshape(S, D)
    # xt[p, st, kc, s] = x[st*128+s, kc*128+p]  (fp16)
    xt = np.ascontiguousarray(
        x.reshape(ST, 128, KC, 128).transpose(3, 0, 2, 1)).astype(np.float16)
    # cs[p, {cos,sin}, st, f] = {cos,sin}[st*128+p, f]
    cosr = np.asarray(cos, np.float32).reshape(ST, 128, HD // 2)
    sinr = np.asarray(sin, np.float32).reshape(ST, 128, HD // 2)
    cs = np.ascontiguousarray(
        np.stack([cosr, sinr], axis=0).transpose(2, 0, 1, 3))
    Wq = np.asarray(Wq, np.float32)
    Wk = np.asarray(Wk, np.float32)
    Wv = np.asarray(Wv, np.float32)
    Wo = np.asarray(Wo, np.float32)
    in_maps = []
    for c in range(N_CORES):
        qs = slice(c * HQ * HD, (c + 1) * HQ * HD)
        ks = slice(c * HD, (c + 1) * HD)
        # wq[p, kc, n] = Wq[kc*128+p, qs][n]
        wq = np.ascontiguousarray(
            Wq[:, qs].reshape(KC, 128, HQ * HD).transpose(1, 0, 2)
        ).astype(np.float16)
        wkv_full = np.concatenate([Wk[:, ks], Wv[:, ks]], axis=1)
        wkv = np.ascontiguousarray(
            wkv_full.reshape(KC, 128, 2 * HD).transpose(1, 0, 2)
        ).astype(np.float16)
        # wo[p, h, n] = Wo[qs][h*128+p, n]
        wo = np.ascontiguousarray(
            Wo[qs, :].reshape(HQ, 128, D).transpose(1, 0, 2)
        ).astype(np.float16)
        in_maps.append({"xt": xt, "cs": cs, "wq": wq, "wkv": wkv, "wo": wo})
    return in_maps


def run(x, cos, sin, Wq, Wk, Wv, Wo, trace=False):
    nc = _get_nc()
    in_maps = make_in_maps(x, cos, sin, Wq, Wk, Wv, Wo)
    res = bass_utils.run_bass_kernel_spmd(
        nc, in_maps, core_ids=list(range(N_CORES)), trace=trace)
    partials = np.stack([res.results[c]["y"] for c in range(N_CORES)])
    out = partials.astype(np.float64).sum(axis=0).astype(np.float32)
    return out.reshape(B, S, D), res


def kernel(x, cos, sin, Wq, Wk, Wv, Wo):
    out, _ = run(x, cos, sin, Wq, Wk, Wv, Wo, trace=False)
    return out


# revision 13
# speedup vs baseline: 1.0997x; 1.0027x over previous
"""Trainium2 Bass kernel for nn_Attention_83743272337693.

Quantized-attention transformer block:
  q/k/v projections -> RoPE(q,k) -> per-token-per-head int8 quantization of
  q,k -> exact int8 score GEMM -> causal softmax -> attn @ v -> o_proj.

Distribution (8 NeuronCores, SPMD): tensor-parallel over heads. Core c owns
query heads 4c..4c+3 and kv head c (GQA group). Wq/Wk/Wv are sharded
column-wise, Wo row-wise; each core computes a full [S, D] partial of the
output and the host sums the 8 partials (the all-reduce).

Numerics strategy (v3):
- projections run in fp16 (full-rate PE path, 10-bit mantissa inputs); the
  int8 round() decisions flip on ~1% of elements vs the fp32 reference,
  which stays well inside the rel-err budget (simulated rel_l2 ~ 5.8e-3,
  HW-measured 6.1e-3 vs tolerance 2e-2).
- quantized q/k are small integers: exact in bf16, so the score GEMM runs
  bf16 at full rate with fp32 PSUM accumulation. k is pre-scaled by its
  dequant factor rk (bf16), q's factor rq folds into the exp scale.
- softmax avoids the row-max pass entirely: a Cauchy-Schwarz bound
  m_hat = rq*||q_int||*cummax_tile||k_s|| is used as the exp bias. probs
  live in bf16, whose exponent range absorbs the bound's overshoot
  (typical p ~ 1e-7; fp16 would denormal-underflow). scores stay in PSUM
  in 512-wide chunks (no S copy to SBUF; chunked exp with a shared bias);
  normalization 1/sum folds into the transpose diag.
- attn@v in bf16, o_proj in fp16, per-core output partial in fp16.
- rounding uses the fp32 magic-constant trick (x + 1.5*2^23 - 1.5*2^23),
  matching np.round (round-half-to-even) exactly.
"""
import numpy as np

import concourse.bass as bass
import concourse.mybir as mybir
from concourse import bacc, bass_utils
from concourse.tile import TileContext
from concourse.masks import make_causal_mask, make_identity

# Problem shape (hardcoded per contract).
B, S, D = 1, 2048, 4096
NH, NKV, HD = 32, 8, 128
N_CORES = 8
HQ = NH // N_CORES          # query heads per core (4)
ST = S // 128               # seq tiles (16)
KC = D // 128               # contraction chunks for projections (32)
SCALE = float(HD) ** -0.5
MAGIC = float(np.float32(1.5 * 2 ** 23))
MASK_VAL = -1.0e10

F32 = mybir.dt.float32
BF16 = mybir.dt.bfloat16
F16 = mybir.dt.float16
ADD = mybir.AluOpType.add
SUB = mybir.AluOpType.subtract
MUL = mybir.AluOpType.mult
MAX = mybir.AluOpType.max
Ident = mybir.ActivationFunctionType.Identity


def build():
    nc = bacc.Bacc("TRN2", target_bir_lowering=False)

    # Host-prepped layouts (see make_in_maps): per-partition-contiguous.
    xt_d = nc.dram_tensor("xt", [128, ST, KC, 128], F16, kind="ExternalInput")
    cs_d = nc.dram_tensor("cs", [128, 2, ST, HD // 2], F32, kind="ExternalInput")
    wq_d = nc.dram_tensor("wq", [128, KC, HQ * HD], F16, kind="ExternalInput")
    wkv_d = nc.dram_tensor("wkv", [128, KC, 2 * HD], F16, kind="ExternalInput")
    wo_d = nc.dram_tensor("wo", [128, HQ, D], F16, kind="ExternalInput")
    y = nc.dram_tensor("y", [S, D], F16, kind="ExternalOutput")

    with TileContext(nc) as tc:
        with (
            tc.tile_pool(name="persist", bufs=1) as persist,
            tc.tile_pool(name="small", bufs=4) as small,
        ):
            # Persistent SBUF state shared by both phases.
            qT = persist.tile([128, HQ, S], BF16, tag="qT")        # 2 MiB
            kTs = persist.tile([128, S], BF16, tag="kTs")          # 512 KiB
            v_sb = persist.tile([128, ST, HD], BF16, tag="v_sb")   # 512 KiB
            rq_sb = persist.tile([128, HQ, ST], F32, tag="rq_sb")  # exp scale
            nbias = persist.tile([128, HQ, ST], F32, tag="nbias")  # exp bias
            qnsq = persist.tile([128, HQ, ST], F32, tag="qnsq")    # ||q_int||^2
            mk2 = persist.tile([1, ST], F32, tag="mk2")            # max||k_s||^2/tile
            mkrun = persist.tile([1, 1], F32, tag="mkrun")         # running max
            ident_bf = persist.tile([128, 128], BF16, tag="ident_bf")
            mask_sb = persist.tile([128, 128], F32, tag="mask_sb")
            ones_sb = persist.tile([1, 128], F32, tag="ones_sb")
            ones_col = persist.tile([128, 1], BF16, tag="ones_col")
            magic_p = persist.tile([128, 1], F32, tag="magic_p")
            magic_n = persist.tile([128, 1], F32, tag="magic_n")
            wo_sb = persist.tile([128, HQ, D], F16, tag="wo_sb")    # 4 MiB
            cs_sb = persist.tile([128, 2, ST, HD // 2], F32, tag="cs_sb")

            make_identity(nc, ident_bf[:])
            make_causal_mask(nc, mask_sb[:], mask_val=MASK_VAL)
            nc.gpsimd.memset(ones_sb[:], 1.0)
            nc.gpsimd.memset(ones_col[:], 1.0)
            nc.gpsimd.memset(magic_p[:], MAGIC)
            nc.gpsimd.memset(magic_n[:], -MAGIC)

            # ---------------- Phase A: projections + rope + quantize ----------
            with (
                tc.tile_pool(name="wproj", bufs=1) as wpool,
                tc.tile_pool(name="xstream", bufs=3) as xpool,
                tc.tile_pool(name="ropebuf", bufs=2) as rpool,
                tc.tile_pool(name="psA", bufs=2, space="PSUM") as psA,
                tc.tile_pool(name="psT", bufs=3, space="PSUM") as psT,
            ):
                wq_sb = wpool.tile([128, KC, HQ * HD], F16, tag="wq_sb")   # 4 MiB
                wkv_sb = wpool.tile([128, KC, 2 * HD], F16, tag="wkv_sb")  # 2 MiB
                # chunked weight loads, smallest first, so the first projection
                # matmuls start as soon as their chunk lands; wo queued last.
                wsched = ([(k, k + 1) for k in range(4)] + [(k, k + 2) for k in range(4, 16, 2)]
                          + [(k, k + 4) for k in range(16, KC, 4)])
                for i, (a, b) in enumerate(wsched):
                    nc.sync.dma_start(wq_sb[:, a:b, :], wq_d.ap()[:, a:b, :])
                    nc.sync.dma_start(wkv_sb[:, a:b, :], wkv_d.ap()[:, a:b, :])
                    if i == 2:
                        nc.sync.dma_start(cs_sb[:], cs_d.ap())
                nc.sync.dma_start(wo_sb[:], wo_d.ap())

                for st in range(ST):
                    xt = xpool.tile([128, KC, 128], F16, tag="xt")
                    for kq in range(0, KC, 8):
                        nc.gpsimd.dma_start(xt[:, kq:kq + 8, :],
                                            xt_d.ap()[:, st, kq:kq + 8, :])

                    ps_q = psA.tile([128, HQ * HD], F32, tag="ps_q")
                    ps_kv = psA.tile([128, 2 * HD], F32, tag="ps_kv")
                    for kc in range(KC):
                        nc.tensor.matmul(ps_q[:], xt[:, kc, :], wq_sb[:, kc, :],
                                         start=(kc == 0), stop=(kc == KC - 1))
                        nc.tensor.matmul(ps_kv[:], xt[:, kc, :], wkv_sb[:, kc, :],
                                         start=(kc == 0), stop=(kc == KC - 1))

                    # RoPE on q heads + k head (DVE, reads PSUM), batched
                    # across heads via broadcast APs on cos/sin.
                    half = HD // 2
                    cos_t = cs_sb[:, 0, st, :]
                    sin_t = cs_sb[:, 1, st, :]
                    cos_q = cos_t.unsqueeze(1).broadcast_to([128, HQ, half])
                    sin_q = sin_t.unsqueeze(1).broadcast_to([128, HQ, half])
                    # rope layout: [128, HQ+1, HD] (heads 0..3 = q, 4 = k)
                    rope = rpool.tile([128, HQ + 1, HD], F32, tag="rope")
                    tmp = rpool.tile([128, HQ + 1, half], F32, tag="tmp")
                    q_r = ps_q[:].rearrange("p (h t f) -> p h t f", h=HQ, t=2)
                    k_r = ps_kv[:, 0:HD].rearrange("p (t f) -> p t f", t=2)
                    # q heads (batched over h)
                    nc.vector.tensor_tensor(rope[:, :HQ, 0:half], q_r[:, :, 0, :], cos_q, op=MUL)
                    nc.vector.tensor_tensor(tmp[:, :HQ, :], q_r[:, :, 1, :], sin_q, op=MUL)
                    nc.vector.tensor_tensor(rope[:, :HQ, 0:half], rope[:, :HQ, 0:half],
                                            tmp[:, :HQ, :], op=SUB)
                    nc.vector.tensor_tensor(rope[:, :HQ, half:HD], q_r[:, :, 0, :], sin_q, op=MUL)
                    nc.vector.tensor_tensor(tmp[:, :HQ, :], q_r[:, :, 1, :], cos_q, op=MUL)
                    nc.vector.tensor_tensor(rope[:, :HQ, half:HD], rope[:, :HQ, half:HD],
                                            tmp[:, :HQ, :], op=ADD)
                    # k head
                    nc.vector.tensor_tensor(rope[:, HQ, 0:half], k_r[:, 0, :], cos_t, op=MUL)
                    nc.vector.tensor_tensor(tmp[:, HQ, :], k_r[:, 1, :], sin_t, op=MUL)
                    nc.vector.tensor_tensor(rope[:, HQ, 0:half], rope[:, HQ, 0:half],
                                            tmp[:, HQ, :], op=SUB)
                    nc.vector.tensor_tensor(rope[:, HQ, half:HD], k_r[:, 0, :], sin_t, op=MUL)
                    nc.vector.tensor_tensor(tmp[:, HQ, :], k_r[:, 1, :], cos_t, op=MUL)
                    nc.vector.tensor_tensor(rope[:, HQ, half:HD], rope[:, HQ, half:HD],
                                            tmp[:, HQ, :], op=ADD)

                    # v: straight cast to bf16 (no rope/quant).
                    nc.scalar.copy(v_sb[:, st, :], ps_kv[:, HD:2 * HD])

                    # Quantize all 5 rope'd heads at once:
                    # am = absmax, s = 127/clip(am,1e-5), int = round(x*s).
                    am = small.tile([128, HQ + 1], F32, tag="am")
                    nc.vector.tensor_reduce(am[:], rope[:], axis=mybir.AxisListType.X,
                                            op=MAX, apply_absolute_value=True)
                    nc.vector.tensor_scalar_max(am[:], am[:], 1e-5)
                    # dequant factors: rq = am*scale/127 (q heads), rk = am/127
                    nc.vector.tensor_scalar_mul(rq_sb[:, :, st], am[:, 0:HQ],
                                                SCALE / 127.0)
                    rk_col = small.tile([128, 1], F32, tag="rk_col")
                    nc.vector.tensor_scalar_mul(rk_col[:], am[:, HQ:HQ + 1],
                                                1.0 / 127.0)
                    sc = small.tile([128, HQ + 1], F32, tag="sc")
                    nc.vector.reciprocal(sc[:], am[:])
                    nc.vector.tensor_scalar_mul(sc[:], sc[:], 127.0)
                    rnd = rpool.tile([128, HQ + 1, HD], F32, tag="rnd")
                    sc_b = sc[:].unsqueeze(2).broadcast_to([128, HQ + 1, HD])
                    nc.gpsimd.tensor_tensor(rnd[:], rope[:], sc_b, op=MUL)
                    rnd_f = rnd[:].rearrange("p h f -> p (h f)")
                    # round via magic constant; the two adds run on ACT
                    nc.scalar.activation(rnd_f, rnd_f, Ident, bias=magic_p[:])
                    qint = rpool.tile([128, (HQ + 1) * HD], BF16, tag="qint")
                    nc.scalar.activation(qint[:], rnd_f, Ident, bias=magic_n[:])

                    # ||q_int||^2 per q head (for the softmax bound)
                    qint_r = qint[:].rearrange("p (h f) -> p h f", h=HQ + 1)
                    sq = rpool.tile([128, HQ, HD], F32, tag="sq")
                    nc.gpsimd.tensor_tensor(sq[:], qint_r[:, 0:HQ, :],
                                            qint_r[:, 0:HQ, :], op=MUL)
                    nc.vector.tensor_reduce(qnsq[:, :, st], sq[:],
                                            axis=mybir.AxisListType.X, op=ADD)

                    # diag(rk): the k-transpose matmul applies the column
                    # dequant scale in the same pass (rk bf16-rounded).
                    diag_rk = rpool.tile([128, 128], BF16, tag="diag_rk")
                    nc.vector.tensor_scalar_mul(diag_rk[:], ident_bf[:],
                                                rk_col[:])

                    # PE transposes of the quantized heads into [hd, seq] layout.
                    sb = slice(st * 128, (st + 1) * 128)
                    for hh in range(HQ + 1):
                        o = hh * HD
                        ps_t = psT.tile([128, 128], F32, tag="ps_t")
                        if hh < HQ:
                            nc.tensor.matmul(ps_t[:], qint[:, o:o + HD],
                                             ident_bf[:])
                            nc.scalar.copy(qT[:, hh, sb], ps_t[:])
                        else:
                            nc.tensor.matmul(ps_t[:], qint[:, o:o + HD],
                                             diag_rk[:])
                            nc.vector.tensor_copy(kTs[:, sb], ps_t[:])
                    # max column norm^2 of this kTs block (softmax bound)
                    ksq = rpool.tile([128, 128], BF16, tag="ksq")
                    nc.vector.tensor_tensor(ksq[:], kTs[:, sb], kTs[:, sb], op=MUL)
                    ps_n = psT.tile([1, 128], F32, tag="ps_t")
                    nc.tensor.matmul(ps_n[:], ones_col[:], ksq[:])
                    nc.vector.tensor_reduce(mk2[:, st:st + 1], ps_n[:],
                                            axis=mybir.AxisListType.X, op=MAX)
                    # softmax bound for this row tile:
                    # nbias[:,h,st] = -rq*||q_int||*sqrt(cummax mk2)
                    if st == 0:
                        nc.vector.tensor_copy(mkrun[:], mk2[:, 0:1])
                    else:
                        nc.vector.tensor_tensor(mkrun[:], mkrun[:],
                                                mk2[:, st:st + 1], op=MAX)
                    mks = small.tile([1, 1], F32, tag="mks")
                    nc.scalar.sqrt(mks[:], mkrun[:])
                    ps_m1 = psT.tile([128, 1], F32, tag="ps_t")
                    nc.tensor.matmul(ps_m1[:], ones_sb[:], mks[:])
                    mkb1 = small.tile([128, 1], F32, tag="mkb1")
                    nc.vector.tensor_copy(mkb1[:], ps_m1[:])
                    qn_t = small.tile([128, HQ], F32, tag="qn_t")
                    nc.scalar.sqrt(qn_t[:], qnsq[:, :, st])
                    rqn_t = small.tile([128, HQ], F32, tag="rqn_t")
                    nc.vector.tensor_tensor(rqn_t[:], rq_sb[:, :, st], qn_t[:],
                                            op=MUL)
                    nc.vector.scalar_tensor_tensor(
                        nbias[:, :, st], rqn_t[:], -1.0,
                        mkb1[:].broadcast_to([128, HQ]), op0=MUL, op1=MUL)

            # ---------------- Phase B: attention + o_proj -----------------
            with (
                tc.tile_pool(name="sbp", bufs=5) as sbp,
                tc.tile_pool(name="pbuf", bufs=2) as pbuf,
                tc.tile_pool(name="obuf", bufs=3) as obuf,
                tc.tile_pool(name="psS", bufs=3, space="PSUM") as psS,
                tc.tile_pool(name="psP", bufs=2, space="PSUM") as psP,
                tc.tile_pool(name="psV", bufs=1, space="PSUM") as psV,
                tc.tile_pool(name="psO", bufs=2, space="PSUM") as psO,
            ):
                for qt in range(ST):
                    K = (qt + 1) * 128
                    nchunk = (K + 511) // 512
                    pT = pbuf.tile([128, HQ, S], BF16, tag="pT")
                    heads = []
                    for h in range(HQ):
                        lhs_q = qT[:, h, qt * 128:(qt + 1) * 128]
                        p_sb = sbp.tile([128, S], BF16, tag="p_sb")
                        ssum = small.tile([128, 4], F32, tag="ssum")
                        rq_h = rq_sb[:, h, qt:qt + 1]
                        nb_h = nbias[:, h, qt:qt + 1]
                        # 512-wide PSUM-resident chunks, shared global bias
                        for ci in range(nchunk):
                            c0 = ci * 512
                            cw = min(K, c0 + 512) - c0
                            ps_S = psS.tile([128, 512], F32, tag="ps_S")
                            nc.tensor.matmul(ps_S[:, 0:cw], lhs_q,
                                             kTs[:, c0:c0 + cw])
                            if ci == nchunk - 1:
                                # causal mask on the diagonal block
                                db = qt * 128 - c0
                                nc.vector.tensor_tensor(
                                    ps_S[:, db:db + 128], ps_S[:, db:db + 128],
                                    mask_sb[:], op=ADD)
                            nc.scalar.activation(
                                p_sb[:, c0:c0 + cw], ps_S[:, 0:cw],
                                mybir.ActivationFunctionType.Exp,
                                bias=nb_h, scale=rq_h,
                                accum_out=ssum[:, ci:ci + 1])
                        w = small.tile([128, 1], F32, tag="w")
                        if nchunk > 1:
                            tot = small.tile([128, 1], F32, tag="tot")
                            nc.vector.tensor_reduce(tot[:], ssum[:, 0:nchunk],
                                                    axis=mybir.AxisListType.X,
                                                    op=ADD)
                            nc.vector.reciprocal(w[:], tot[:])
                        else:
                            nc.vector.reciprocal(w[:], ssum[:, 0:1])
                        diag = sbp.tile([128, 128], BF16, tag="diag")
                        nc.vector.tensor_scalar_mul(diag[:], ident_bf[:], w[:])
                        heads.append((p_sb, diag))
                    # p^T (scaled by 1/sum) via PE, two blocks per bank;
                    # separate stream so score matmuls don't thrash stationary
                    for h in range(HQ):
                        p_sb, diag = heads[h]
                        for kc in range(0, qt + 1, 2):
                            kn = min(2, qt + 1 - kc)
                            ps_p = psP.tile([128, 256], F32, tag="ps_p")
                            for j in range(kn):
                                nc.tensor.matmul(
                                    ps_p[:, j * 128:(j + 1) * 128],
                                    p_sb[:, (kc + j) * 128:(kc + j + 1) * 128],
                                    diag[:])
                            if kc % 4 < 2:
                                nc.vector.tensor_copy(
                                    pT[:, h, kc * 128:(kc + kn) * 128],
                                    ps_p[:, :kn * 128])
                            else:
                                nc.scalar.copy(
                                    pT[:, h, kc * 128:(kc + kn) * 128],
                                    ps_p[:, :kn * 128])
                    # attn @ v for all 4 heads at once (N=512 moving)
                    ps_oh = psV.tile([128, HQ * 128], F32, tag="ps_oh")
                    for kc in range(qt + 1):
                        nc.tensor.matmul(
                            ps_oh[:], v_sb[:, kc, :],
                            pT[:, :, kc * 128:(kc + 1) * 128],
                            start=(kc == 0), stop=(kc == qt))
                    ohT = pbuf.tile([128, HQ * 128], F16, tag="ohT")
                    for h in range(HQ):
                        hb = slice(h * 128, (h + 1) * 128)
                        if h % 2 == 0:
                            nc.scalar.copy(ohT[:, hb], ps_oh[:, hb])
                        else:
                            nc.vector.tensor_copy(ohT[:, hb], ps_oh[:, hb])
                    # o_proj for this q-tile: accumulate the 4 heads.
                    for b in range(D // 512):
                        ps_O = psO.tile([128, 512], F32, tag="ps_O")
                        for h in range(HQ):
                            nc.tensor.matmul(
                                ps_O[:], ohT[:, h * 128:(h + 1) * 128],
                                wo_sb[:, h, b * 512:(b + 1) * 512],
                                start=(h == 0), stop=(h == HQ - 1))
                        out_t = obuf.tile([128, 512], F16, tag="out_t")
                        if b % 2 == 0:
                            nc.vector.tensor_copy(out_t[:], ps_O[:])
                        else:
                            nc.scalar.copy(out_t[:], ps_O[:])
                        nc.gpsimd.dma_start(
                            y.ap()[qt * 128:(qt + 1) * 128,
                                   b * 512:(b + 1) * 512], out_t[:])

    nc.finalize()
    return nc


_NC_CACHE = None


def _get_nc():
    global _NC_CACHE
    if _NC_CACHE is None:
        _NC_CACHE = build()
    return _NC_CACHE


def make_in_maps(x, cos, sin, Wq, Wk, Wv, Wo):
    """Shard the full inputs into the 8 per-core input maps."""
    x = np.asarray(x, np.float32).reshape(S, D)
    # xt[p, st, kc, s] = x[st*128+s, kc*128+p]  (fp16)
    xt = np.ascontiguousarray(
        x.reshape(ST, 128, KC, 128).transpose(3, 0, 2, 1)).astype(np.float16)
    # cs[p, {cos,sin}, st, f] = {cos,sin}[st*128+p, f]
    cosr = np.asarray(cos, np.float32).reshape(ST, 128, HD // 2)
    sinr = np.asarray(sin, np.float32).reshape(ST, 128, HD // 2)
    cs = np.ascontiguousarray(
        np.stack([cosr, sinr], axis=0).transpose(2, 0, 1, 3))
    Wq = np.asarray(Wq, np.float32)
    Wk = np.asarray(Wk, np.float32)
    Wv = np.asarray(Wv, np.float32)
    Wo = np.asarray(Wo, np.float32)
    in_maps = []
    for c in range(N_CORES):
        qs = slice(c * HQ * HD, (c + 1) * HQ * HD)
        ks = slice(c * HD, (c + 1) * HD)
        # wq[p, kc, n] = Wq[kc*128+p, qs][n]
        wq = np.ascontiguousarray(
            Wq[:, qs].reshape(KC, 128, HQ * HD).transpose(1, 0, 2)
        ).astype(np.float16)
        wkv_full = np.concatenate([Wk[:, ks], Wv[:, ks]], axis=1)
        wkv = np.ascontiguousarray(
            wkv_full.reshape(KC, 128, 2 * HD).transpose(1, 0, 2)
        ).astype(np.float16)
        # wo[p, h, n] = Wo[qs][h*128+p, n]
        wo = np.ascontiguousarray(
            Wo[qs, :].reshape(HQ, 128, D).transpose(1, 0, 2)
        ).astype(np.float16)
        in_maps.append({"xt": xt, "cs": cs, "wq": wq, "wkv": wkv, "wo": wo})
    return in_maps


def run(x, cos, sin, Wq, Wk, Wv, Wo, trace=False):
    nc = _get_nc()
    in_maps = make_in_maps(x, cos, sin, Wq, Wk, Wv, Wo)
    res = bass_utils.run_bass_kernel_spmd(
        nc, in_maps, core_ids=list(range(N_CORES)), trace=trace)
    partials = np.stack([res.results[c]["y"] for c in range(N_CORES)])
    out = partials.astype(np.float64).sum(axis=0).astype(np.float32)
    return out.reshape(B, S, D), res


def kernel(x, cos, sin, Wq, Wk, Wv, Wo):
    out, _ = run(x, cos, sin, Wq, Wk, Wv, Wo, trace=False)
    return out
